# revision 11
# baseline (speedup 1.0000x reference)
"""Multi-head attention (B=2, S=2048, D=1024, H=16, dk=64) on 8 trn2 cores.

Sharding: core c handles batch b=c//4 and 4 heads g=c%4 (heads 4g..4g+3).
Each core computes its heads' Q/K/V projections, attention, and a partial
output projection; the host sums the 4 partials per batch.

v2 — transposed-scores layout (k on partitions), no DMA transpose:
  - qhT/khT [128 = 2 heads x 64 dk, S] bf16; vh [128 k, 16 kt, 256 hd] bf16.
  - scoresT [128 k, 512 q] per (head, kt): lhsT = khT slice (K=64 rows at
    partition base po), rhs = qhT slice. The two heads of a pair are
    row-packed (tile_position (0,0)/(64,0)) and run concurrently on the PE.
  - exp on ScalarE straight from PSUM [128, 2, 512] (one kt, both heads) to
    bf16 expT in SBUF. No accum_out, no max subtraction (scores ~ N(0,1)).
  - PV contracts k on partitions: lhsT = vh slice [128, 64], rhs = expT
    slice, the two heads col-packed (tile_position (0,0)/(0,64)) into one
    ctx PSUM bank [128 = 2x64 dk, 512 q], accumulated over the 16 kt.
  - softmax denominator: bf16 add-tree over kt on DVE (in-place halving),
    cross-partition sum on GpSimd (tensor_reduce axis=C), reciprocal on
    DVE, partition-broadcast via K=1 PE matmuls (ones column) into a PSUM
    bank; the normalize-multiply is fused into the ctx PSUM->SBUF copy.
  - out projection reads ctxT (already [hd, q]) against resident wo.
  - bk/bv/bo never touch the device: bk shifts scores by a per-row constant
    (softmax-invariant), bv/bo fold into a host-side correction, bq is a
    rank-1 matmul on-device (x 1/sqrt(dk), folded into wq/bq on the host).

Emission is software-pipelined so ScalarE (the binding engine, ~130us of
exp) streams: combo ci emits scores(ci) + PV(ci-2) + filler quanta
(projection chunks / out-proj quanta) per kt step. Scores only need khT at
kt-tile granularity, so combo 0 starts right after the first K/Q chunks.
PSUM budget: sc(4 banks) + ctx(1) + bcast(1) + qk-proj(1) + v-proj(1) = 8;
the out-proj pool (2) opens after the proj pools close.
"""

import sys

for _p in ("/opt/trn_rl_repo",):
    if _p not in sys.path:
        sys.path.insert(0, _p)

from contextlib import ExitStack

import ml_dtypes
import numpy as np

import concourse.bass as bass
import concourse.bass_isa as bass_isa
import concourse.bacc as bacc_mod
import concourse.mybir as mybir
import concourse.tile as tile
from concourse.bass_utils import run_bass_kernel_spmd

BF16 = mybir.dt.bfloat16
F32 = mybir.dt.float32
AF = mybir.ActivationFunctionType
ALU = mybir.AluOpType
AXC = mybir.AxisListType.C

B, S, D = 2, 2048, 1024
NCORES = 8
HLOC = 4          # heads per core
DK = 64
HD = HLOC * DK    # local head dims = 256
KT_D = D // 128   # 8 k-tiles over the model dim
NST = S // 128    # 16 tiles over sequence (kv)
NQB = 4           # q blocks of 512
NCH = 4           # proj output chunks of 512 over S
LAG = 2           # PV trails scores by LAG combos


def build_nc() -> bass.Bass:
    nc = bacc_mod.Bacc()

    xqT = nc.dram_tensor("xqT", [D, S], BF16, kind="ExternalInput")
    xkT = nc.dram_tensor("xkT", [D, S], BF16, kind="ExternalInput")
    xvT = nc.dram_tensor("xvT", [D, S], BF16, kind="ExternalInput")
    wqT = nc.dram_tensor("wqT", [D, HD], BF16, kind="ExternalInput")
    wkT = nc.dram_tensor("wkT", [D, HD], BF16, kind="ExternalInput")
    wvT = nc.dram_tensor("wvT", [D, HD], BF16, kind="ExternalInput")
    woT = nc.dram_tensor("woT", [HD, D], BF16, kind="ExternalInput")
    bqd = nc.dram_tensor("bq", [HD], BF16, kind="ExternalInput")
    out = nc.dram_tensor("out_partial", [S, D], F32, kind="ExternalOutput")

    with tile.TileContext(nc) as tc, ExitStack() as ctx:
        const = ctx.enter_context(tc.tile_pool(name="const", bufs=1))
        persist = ctx.enter_context(tc.tile_pool(name="persist", bufs=1))

        wq_s = const.tile([128, KT_D, HD], BF16, tag="wq")
        wk_s = const.tile([128, KT_D, HD], BF16, tag="wk")
        wv_s = const.tile([128, KT_D, HD], BF16, tag="wv")
        wo_s = const.tile([128, 2, D], BF16, tag="wo")
        bq_s = const.tile([1, HD], BF16, tag="bq")
        ones_row = const.tile([1, 512], BF16, tag="ones")
        ones_col = const.tile([1, DK], BF16, tag="onesc")
        nc.vector.memset(ones_row, 1.0)
        nc.vector.memset(ones_col, 1.0)

        # sync queue: wk, wq, bq, xq chunks..., wv, wo. gpsimd queue: xk
        # chunks, xv chunks, output stores.
        nc.sync.dma_start(wk_s, wkT.rearrange("(t p) n -> p t n", p=128))
        nc.sync.dma_start(wq_s, wqT.rearrange("(t p) n -> p t n", p=128))
        nc.sync.dma_start(bq_s, bqd.rearrange("(o n) -> o n", o=1))

        qhT = [persist.tile([128, S], BF16, tag=f"qhT{m}", name=f"qhT{m}") for m in range(2)]
        khT = [persist.tile([128, S], BF16, tag=f"khT{m}", name=f"khT{m}") for m in range(2)]
        vh_s = persist.tile([128, NST, HD], BF16, tag="vh")
        ctxT = [persist.tile([128, S], BF16, tag=f"ctxT{m}", name=f"ctxT{m}") for m in range(2)]

        xkp = ctx.enter_context(tc.tile_pool(name="xkload", bufs=2))
        xqp = ctx.enter_context(tc.tile_pool(name="xqload", bufs=2))
        xvp = ctx.enter_context(tc.tile_pool(name="xvload", bufs=2))
        xk_t, xq_t, xv_t = {}, {}, {}

        def load_xk(c):
            t = xkp.tile([128, KT_D, 512], BF16, tag="xk", name=f"xk{c}")
            nc.gpsimd.dma_start(
                t, xkT[:, c * 512 : (c + 1) * 512].rearrange("(t p) n -> p t n", p=128)
            )
            xk_t[c] = t

        def load_xq(c):
            t = xqp.tile([128, KT_D, 512], BF16, tag="xq", name=f"xq{c}")
            nc.sync.dma_start(
                t, xqT[:, c * 512 : (c + 1) * 512].rearrange("(t p) n -> p t n", p=128)
            )
            xq_t[c] = t

        def load_xv(pr):
            t = xvp.tile([128, KT_D, 256], BF16, tag="xv", name=f"xv{pr}")
            nc.gpsimd.dma_start(
                t, xvT[:, pr * 256 : (pr + 1) * 256].rearrange("(t p) n -> p t n", p=128)
            )
            xv_t[pr] = t

        load_xk(0)
        load_xk(1)
        load_xq(0)
        load_xq(1)
        load_xv(0)
        load_xv(1)
        nc.sync.dma_start(wv_s, wvT.rearrange("(t p) n -> p t n", p=128))
        nc.sync.dma_start(wo_s, woT.rearrange("(t p) n -> p t n", p=128))

        # softmax stat tiles. stats(ci) [combo ci+1 kt1] and bcast(ci)
        # [combo ci+1 kt13] use them strictly before stats(ci+1) rewrites.
        t4 = persist.tile([128, 4, 512], BF16, tag="t4")
        # parity-indexed: stats(ci) [combo ci+1 kt1] must not clobber the
        # recip that bcast_mm(ci-1) reads later the same combo [kt13]
        recip = [
            [persist.tile([1, 512], BF16, tag=f"rec{p}{e}", name=f"rec{p}{e}") for e in range(2)]
            for p in range(2)
        ]
        bcs = persist.tile([128, 512], BF16, tag="bcs")

        expT_pool = ctx.enter_context(tc.tile_pool(name="expT", bufs=LAG + 1))
        ostage_pool = ctx.enter_context(tc.tile_pool(name="ostage", bufs=2))

        sc_pool = ctx.enter_context(tc.tile_pool(name="sc_psum", bufs=2, space="PSUM"))
        cb_pool = ctx.enter_context(tc.tile_pool(name="cb_psum", bufs=1, space="PSUM"))

        proj_stack = ExitStack()
        qkp = proj_stack.enter_context(tc.tile_pool(name="qk_psum", bufs=1, space="PSUM"))

        # ---------------- emission helpers -------------------------------
        def qk_chunk(dst, w_s, xt, m, c, bias, after=None):
            ps = qkp.tile([128, 512], F32, tag="proj")
            for kt in range(KT_D):
                nc.tensor.matmul(
                    ps,
                    lhsT=w_s[:, kt, m * 128 : (m + 1) * 128],
                    rhs=xt[c][:, kt, :],
                    start=(kt == 0),
                    stop=(kt == KT_D - 1) and not bias,
                )
            if bias:
                nc.tensor.matmul(
                    ps,
                    lhsT=bq_s[0:1, m * 128 : (m + 1) * 128],
                    rhs=ones_row[0:1, :],
                    start=False,
                    stop=True,
                )
            nc.vector.tensor_copy(dst[:, c * 512 : (c + 1) * 512], ps)
            if after is not None:
                after()

        def v_chunk(st):
            ps = qkp.tile([128, 512], F32, tag="proj", name=f"vproj{st}")
            pr, half = st // 2, st % 2
            for kt in range(KT_D):
                nc.tensor.matmul(
                    ps[:, 0:HD],
                    lhsT=xv_t[pr][:, kt, half * 128 : (half + 1) * 128],
                    rhs=wv_s[:, kt, :],
                    start=(kt == 0),
                    stop=(kt == KT_D - 1),
                )
            nc.vector.tensor_copy(vh_s[:, st, :], ps[:, 0:HD])
            # both halves of pair pr consumed -> its buffer may be reused
            if half == 1 and pr + 2 < NST // 2:
                load_xv(pr + 2)

        combos = [(qb, hp) for qb in range(NQB) for hp in range(2)]
        NC = len(combos)
        expT, ctx_ps = {}, {}

        def softmax_stats(ci):
            """DVE add-tree + GpSimd partition-sum + reciprocal, both heads."""
            et = expT[ci]
            with nc.allow_low_precision("bf16 softmax denominator tree"):
                for e in range(2):
                    nc.vector.tensor_add(t4, et[:, 0:4, e, :], et[:, 4:8, e, :])
                    nc.vector.tensor_add(t4, t4, et[:, 8:12, e, :])
                    nc.vector.tensor_add(t4, t4, et[:, 12:16, e, :])
                    nc.vector.tensor_add(t4[:, 0:2], t4[:, 0:2], t4[:, 2:4])
                    nc.vector.tensor_add(t4[:, 0:1], t4[:, 0:1], t4[:, 1:2])
                    nc.gpsimd.partition_all_reduce(
                        t4[:, 1, :], t4[:, 0, :], 128, bass_isa.ReduceOp.add
                    )
                    nc.vector.reciprocal(recip[ci % 2][e], t4[0:1, 1, :])

        def bcast_mm(ci):
            bc = cb_pool.tile([128, 512], F32, tag="bcast", name=f"bc{ci}")
            for e in range(2):
                nc.tensor.matmul(
                    bc[e * 64 : (e + 1) * 64, :],
                    lhsT=ones_col[0:1, :],
                    rhs=recip[ci % 2][e][0:1, :],
                    start=True,
                    stop=True,
                    tile_position=(0, e * 64),
                )
            # DVE may read only one PSUM operand in the normalize multiply
            nc.vector.tensor_copy(bcs, bc)
            return bcs

        def normalize(ci, bc):
            qb, hp = combos[ci]
            cp = ctx_ps[ci]
            for e in range(2):
                sl = slice(e * 64, (e + 1) * 64)
                nc.vector.tensor_mul(
                    ctxT[hp][sl, qb * 512 : (qb + 1) * 512], cp[sl, e, :], bc[sl, :]
                )
            del expT[ci], ctx_ps[ci]

        def pv_step(ci, kt):
            qb, hp = combos[ci]
            if kt == 0:
                ctx_ps[ci] = cb_pool.tile([128, 2, 512], F32, tag="ctx", name=f"ctx{ci}")
            for e in range(2):
                nc.tensor.matmul(
                    ctx_ps[ci][e * 64 : (e + 1) * 64, e, :],
                    lhsT=vh_s[:, kt, (hp * 2 + e) * 64 : (hp * 2 + e + 1) * 64],
                    rhs=expT[ci][:, kt, e, :],
                    start=(kt == 0),
                    stop=(kt == NST - 1),
                    tile_position=(0, e * 64),
                )

        outp_pool = [None]

        def outproj_quantum(qb, stl):
            st = qb * 4 + stl
            outp = outp_pool[0]
            for nb in range(2):
                ps = outp.tile([128, 512], F32, tag="op", name=f"op{st}_{nb}")
                for kt2 in range(2):
                    nc.tensor.matmul(
                        ps,
                        lhsT=ctxT[kt2][:, st * 128 : (st + 1) * 128],
                        rhs=wo_s[:, kt2, nb * 512 : (nb + 1) * 512],
                        start=(kt2 == 0),
                        stop=(kt2 == 1),
                    )
                ob = ostage_pool.tile([128, 512], F32, tag="ob")
                nc.vector.tensor_copy(ob, ps)
                nc.gpsimd.dma_start(
                    out[st * 128 : (st + 1) * 128, nb * 512 : (nb + 1) * 512], ob
                )

        # ---------------- prologue ---------------------------------------
        # Chunk 0 of khT (both halves) + chunk 0 of qhT (both halves):
        # combo 0's first 4 score kt-tiles only need these.
        qk_chunk(khT[0], wk_s, xk_t, 0, 0, bias=False)
        qk_chunk(khT[1], wk_s, xk_t, 1, 0, bias=False, after=lambda: load_xk(2))
        qk_chunk(qhT[0], wq_s, xq_t, 0, 0, bias=True)
        qk_chunk(qhT[1], wq_s, xq_t, 1, 0, bias=True, after=lambda: load_xq(2))

        # Filler schedule: {(ci, kt): [thunk, ...]}.  K chunk c feeds score
        # kt-tiles 4c..4c+3, so chunk c is emitted well before step 4c.
        fill = {}

        def F(ci, kt, fn):
            fill.setdefault((ci, kt), []).append(fn)

        F(0, 0, lambda: qk_chunk(khT[0], wk_s, xk_t, 0, 1, bias=False))
        F(0, 1, lambda: qk_chunk(khT[1], wk_s, xk_t, 1, 1, bias=False,
                                 after=lambda: load_xk(3)))
        F(0, 4, lambda: qk_chunk(khT[0], wk_s, xk_t, 0, 2, bias=False))
        F(0, 5, lambda: qk_chunk(khT[1], wk_s, xk_t, 1, 2, bias=False))
        F(0, 8, lambda: qk_chunk(khT[0], wk_s, xk_t, 0, 3, bias=False))
        F(0, 9, lambda: qk_chunk(khT[1], wk_s, xk_t, 1, 3, bias=False))
        F(0, 12, lambda: qk_chunk(qhT[0], wq_s, xq_t, 0, 1, bias=True))
        F(0, 14, lambda: qk_chunk(qhT[1], wq_s, xq_t, 1, 1, bias=True,
                                  after=lambda: load_xq(3)))
        # V projection: st 0..7 during ci1, st 8..15 during ci2 (PV(0) at
        # combo 2 consumes st_kt at step kt; st8 lands at ci2 step 0).
        for i, st in enumerate(range(8)):
            F(1, 2 * i, lambda st=st: v_chunk(st))
        for i, st in enumerate(range(8, NST)):
            F(2, 2 * i, lambda st=st: v_chunk(st))
        # remaining q chunks
        F(1, 11, lambda: qk_chunk(qhT[0], wq_s, xq_t, 0, 2, bias=True))
        F(1, 13, lambda: qk_chunk(qhT[1], wq_s, xq_t, 1, 2, bias=True))
        F(2, 11, lambda: qk_chunk(qhT[0], wq_s, xq_t, 0, 3, bias=True))
        F(2, 13, lambda: qk_chunk(qhT[1], wq_s, xq_t, 1, 3, bias=True))
        # out-projection quanta: qb0 -> ci4, qb1 -> ci6 (qb2/qb3 in tail)
        for i in range(4):
            F(4, 2 * i + 5, lambda i=i: outproj_quantum(0, i))
            F(6, 2 * i + 5, lambda i=i: outproj_quantum(1, i))

        # ---------------- main pipeline ----------------------------------
        pend_bc = {}
        for ci, (qb, hp) in enumerate(combos):
            et = expT_pool.tile([128, NST, 2, 512], BF16, tag="expT", name=f"expT{ci}")
            expT[ci] = et
            for kt in range(NST):
                sc = sc_pool.tile([128, 2, 512], F32, tag="sc")
                for e in range(2):
                    po = e * 64
                    nc.tensor.matmul(
                        sc[:, e, :],
                        lhsT=khT[hp][po : po + 64, kt * 128 : (kt + 1) * 128],
                        rhs=qhT[hp][po : po + 64, qb * 512 : (qb + 1) * 512],
                        start=True,
                        stop=True,
                        tile_position=(po, 0),
                    )
                nc.scalar.activation(et[:, kt, :, :], sc, AF.Exp)
                if ci >= LAG:
                    pv_step(ci - LAG, kt)
                for fn in fill.pop((ci, kt), ()):
                    fn()
                if kt == 1 and ci >= 1:
                    softmax_stats(ci - 1)
                if kt == 13 and ci >= LAG:
                    pend_bc[ci - LAG] = bcast_mm(ci - LAG)
                if ci == 3 and kt == 0:
                    # all projection work emitted; swap proj PSUM for out-proj
                    proj_stack.close()
                    outp_pool[0] = ctx.enter_context(
                        tc.tile_pool(name="out_psum", bufs=1, space="PSUM")
                    )
            if ci >= LAG:
                normalize(ci - LAG, pend_bc.pop(ci - LAG))

        # ---------------- tail -------------------------------------------
        for ci in (NC - 2, NC - 1):
            for kt in range(NST):
                pv_step(ci, kt)
                if ci == NC - 1 and kt in (5, 7, 9, 11):
                    outproj_quantum(2, (kt - 5) // 2)
            if ci == NC - 1:
                softmax_stats(ci)
            normalize(ci, bcast_mm(ci))
        for stl in range(4):
            outproj_quantum(3, stl)

    nc.compile()
    return nc


_CACHE: dict = {}


def _get_nc() -> bass.Bass:
    if "nc" not in _CACHE:
        _CACHE["nc"] = build_nc()
    return _CACHE["nc"]


def _bf16(x: np.ndarray) -> np.ndarray:
    return np.ascontiguousarray(x).astype(ml_dtypes.bfloat16)


def make_in_maps(q, k, v, wq, bq, wk, bk, wv, bv, wo, bo):
    scale = np.float32(1.0 / np.sqrt(DK))
    in_maps = []
    for c in range(NCORES):
        b, g = divmod(c, 4)
        hh = g * HD
        in_maps.append(
            {
                "xqT": _bf16(np.asarray(q[b], np.float32).T),
                "xkT": _bf16(np.asarray(k[b], np.float32).T),
                "xvT": _bf16(np.asarray(v[b], np.float32).T),
                "wqT": _bf16(np.asarray(wq[hh : hh + HD], np.float32).T * scale),
                "wkT": _bf16(np.asarray(wk[hh : hh + HD], np.float32).T),
                "wvT": _bf16(np.asarray(wv[hh : hh + HD], np.float32).T),
                "woT": _bf16(np.asarray(wo[:, hh : hh + HD], np.float32).T),
                "bq": _bf16(np.asarray(bq[hh : hh + HD], np.float32) * scale),
            }
        )
    return in_maps


def assemble(results, bv, bo, wo) -> np.ndarray:
    out = np.zeros((B, S, D), np.float32)
    for c in range(NCORES):
        out[c // 4] += np.asarray(results[c]["out_partial"], np.float32)
    corr = np.asarray(bv, np.float32) @ np.asarray(wo, np.float32).T + np.asarray(
        bo, np.float32
    )
    out += corr[None, None, :]
    return out


def kernel(q, k, v, wq, bq, wk, bk, wv, bv, wo, bo) -> np.ndarray:
    nc = _get_nc()
    in_maps = make_in_maps(q, k, v, wq, bq, wk, bk, wv, bv, wo, bo)
    res = run_bass_kernel_spmd(nc, in_maps, list(range(NCORES))).results
    return assemble(res, bv, bo, wo)


# revision 12
# speedup vs baseline: 1.2764x; 1.2764x over previous
"""Multi-head attention (B=2, S=2048, D=1024, H=16, dk=64) on 8 trn2 cores.

Sharding: core c handles batch b=c//4 and 4 heads g=c%4 (heads 4g..4g+3).
Each core computes its heads' Q/K/V projections, attention, and a partial
output projection; the host sums the 4 partials per batch.

v2 — transposed-scores layout (k on partitions), no DMA transpose:
  - qhT/khT [128 = 2 heads x 64 dk, S] bf16; vh [128 k, 16 kt, 256 hd] bf16.
  - scoresT [128 k, 512 q] per (head, kt): lhsT = khT slice (K=64 rows at
    partition base po), rhs = qhT slice. The two heads of a pair are
    row-packed (tile_position (0,0)/(64,0)) and run concurrently on the PE.
  - exp on ScalarE straight from PSUM [128, 2, 512] (one kt, both heads) to
    bf16 expT in SBUF. No accum_out, no max subtraction (scores ~ N(0,1)).
  - PV contracts k on partitions: lhsT = vh slice [128, 64], rhs = expT
    slice, the two heads col-packed (tile_position (0,0)/(0,64)) into one
    ctx PSUM bank [128 = 2x64 dk, 512 q], accumulated over the 16 kt.
  - softmax denominator: bf16 add-tree over kt on DVE (in-place halving),
    cross-partition sum on GpSimd (tensor_reduce axis=C), reciprocal on
    DVE, partition-broadcast via K=1 PE matmuls (ones column) into a PSUM
    bank; the normalize-multiply is fused into the ctx PSUM->SBUF copy.
  - out projection reads ctxT (already [hd, q]) against resident wo.
  - bk/bv/bo never touch the device: bk shifts scores by a per-row constant
    (softmax-invariant), bv/bo fold into a host-side correction, bq is a
    rank-1 matmul on-device (x 1/sqrt(dk), folded into wq/bq on the host).

Emission is software-pipelined so ScalarE (the binding engine, ~130us of
exp) streams: combo ci emits scores(ci) + PV(ci-2) + filler quanta
(projection chunks / out-proj quanta) per kt step. Scores only need khT at
kt-tile granularity, so combo 0 starts right after the first K/Q chunks.
PSUM budget: sc(4 banks) + ctx(1) + bcast(1) + qk-proj(1) + v-proj(1) = 8;
the out-proj pool (2) opens after the proj pools close.
"""

import sys

for _p in ("/opt/trn_rl_repo",):
    if _p not in sys.path:
        sys.path.insert(0, _p)

from contextlib import ExitStack

import ml_dtypes
import numpy as np

import concourse.bass as bass
import concourse.bass_isa as bass_isa
import concourse.bacc as bacc_mod
import concourse.mybir as mybir
import concourse.tile as tile
from concourse.bass_utils import run_bass_kernel_spmd

BF16 = mybir.dt.bfloat16
F32 = mybir.dt.float32
AF = mybir.ActivationFunctionType
ALU = mybir.AluOpType
AXC = mybir.AxisListType.C

B, S, D = 2, 2048, 1024
NCORES = 8
HLOC = 4          # heads per core
DK = 64
HD = HLOC * DK    # local head dims = 256
KT_D = D // 128   # 8 k-tiles over the model dim
NST = S // 128    # 16 tiles over sequence (kv)
NQB = 4           # q blocks of 512
NCH = 4           # proj output chunks of 512 over S
LAG = 2           # PV trails scores by LAG combos


def build_nc() -> bass.Bass:
    nc = bacc_mod.Bacc()

    xqT = nc.dram_tensor("xqT", [D, S], BF16, kind="ExternalInput")
    xkT = nc.dram_tensor("xkT", [D, S], BF16, kind="ExternalInput")
    xvT = nc.dram_tensor("xvT", [D, S], BF16, kind="ExternalInput")
    wqT = nc.dram_tensor("wqT", [D, HD], BF16, kind="ExternalInput")
    wkT = nc.dram_tensor("wkT", [D, HD], BF16, kind="ExternalInput")
    wvT = nc.dram_tensor("wvT", [D, HD], BF16, kind="ExternalInput")
    woT = nc.dram_tensor("woT", [HD, D], BF16, kind="ExternalInput")
    bqd = nc.dram_tensor("bq", [HD], BF16, kind="ExternalInput")
    out = nc.dram_tensor("out_partial", [S, D], F32, kind="ExternalOutput")

    with tile.TileContext(nc) as tc, ExitStack() as ctx:
        const = ctx.enter_context(tc.tile_pool(name="const", bufs=1))
        persist = ctx.enter_context(tc.tile_pool(name="persist", bufs=1))

        wq_s = const.tile([128, KT_D, HD], BF16, tag="wq")
        wk_s = const.tile([128, KT_D, HD], BF16, tag="wk")
        wv_s = const.tile([128, KT_D, HD], BF16, tag="wv")
        wo_s = const.tile([128, 2, D], BF16, tag="wo")
        bq_s = const.tile([1, HD], BF16, tag="bq")
        ones_row = const.tile([1, 512], BF16, tag="ones")
        ones_col = const.tile([1, DK], BF16, tag="onesc")
        nc.vector.memset(ones_row, 1.0)
        nc.vector.memset(ones_col, 1.0)

        # sync queue: wk, wq, bq, xq chunks..., wv, wo. gpsimd queue: xk
        # chunks, xv chunks, output stores.
        nc.sync.dma_start(wk_s, wkT.rearrange("(t p) n -> p t n", p=128))
        nc.sync.dma_start(wq_s, wqT.rearrange("(t p) n -> p t n", p=128))
        nc.sync.dma_start(bq_s, bqd.rearrange("(o n) -> o n", o=1))

        qhT = [persist.tile([128, S], BF16, tag=f"qhT{m}", name=f"qhT{m}") for m in range(2)]
        khT = [persist.tile([128, S], BF16, tag=f"khT{m}", name=f"khT{m}") for m in range(2)]
        vh_s = persist.tile([128, NST, HD], BF16, tag="vh")
        ctxT = [persist.tile([128, S], BF16, tag=f"ctxT{m}", name=f"ctxT{m}") for m in range(2)]

        xkp = ctx.enter_context(tc.tile_pool(name="xkload", bufs=2))
        xqp = ctx.enter_context(tc.tile_pool(name="xqload", bufs=2))
        xvp = ctx.enter_context(tc.tile_pool(name="xvload", bufs=2))
        xk_t, xq_t, xv_t = {}, {}, {}

        def load_xk(c):
            t = xkp.tile([128, KT_D, 512], BF16, tag="xk", name=f"xk{c}")
            nc.gpsimd.dma_start(
                t, xkT[:, c * 512 : (c + 1) * 512].rearrange("(t p) n -> p t n", p=128)
            )
            xk_t[c] = t

        def load_xq(c):
            t = xqp.tile([128, KT_D, 512], BF16, tag="xq", name=f"xq{c}")
            nc.sync.dma_start(
                t, xqT[:, c * 512 : (c + 1) * 512].rearrange("(t p) n -> p t n", p=128)
            )
            xq_t[c] = t

        def load_xv(pr):
            t = xvp.tile([128, KT_D, 256], BF16, tag="xv", name=f"xv{pr}")
            nc.gpsimd.dma_start(
                t, xvT[:, pr * 256 : (pr + 1) * 256].rearrange("(t p) n -> p t n", p=128)
            )
            xv_t[pr] = t

        load_xk(0)
        load_xk(1)
        load_xq(0)
        load_xq(1)
        load_xv(0)
        load_xv(1)
        nc.sync.dma_start(wv_s, wvT.rearrange("(t p) n -> p t n", p=128))
        nc.sync.dma_start(wo_s, woT.rearrange("(t p) n -> p t n", p=128))

        # softmax stat tiles. stats(ci) [combo ci+1 kt1] and bcast(ci)
        # [combo ci+1 kt13] use them strictly before stats(ci+1) rewrites.
        t4 = persist.tile([128, 8, 512], BF16, tag="t4")
        denf = persist.tile([128, 512], F32, tag="denf")
        recipf = persist.tile([1, 512], F32, tag="recipf")
        # parity-indexed: stats(ci) [combo ci+1 kt1] must not clobber the
        # recip that bcast_mm(ci-1) reads later the same combo [kt13]
        recip = [
            [persist.tile([1, 512], BF16, tag=f"rec{p}{e}", name=f"rec{p}{e}") for e in range(2)]
            for p in range(2)
        ]
        bcs = persist.tile([128, 512], BF16, tag="bcs")

        expT_pool = ctx.enter_context(tc.tile_pool(name="expT", bufs=LAG + 1))
        ostage_pool = ctx.enter_context(tc.tile_pool(name="ostage", bufs=2))

        sc_pool = ctx.enter_context(tc.tile_pool(name="sc_psum", bufs=1, space="PSUM"))
        cb_pool = ctx.enter_context(tc.tile_pool(name="cb_psum", bufs=1, space="PSUM"))

        proj_stack = ExitStack()
        qkp = proj_stack.enter_context(tc.tile_pool(name="qk_psum", bufs=1, space="PSUM"))

        # ---------------- emission helpers -------------------------------
        def qk_chunk(dst, w_s, xt, m, c, bias, after=None):
            ps = qkp.tile([128, 512], F32, tag="proj")
            for kt in range(KT_D):
                nc.tensor.matmul(
                    ps,
                    lhsT=w_s[:, kt, m * 128 : (m + 1) * 128],
                    rhs=xt[c][:, kt, :],
                    start=(kt == 0),
                    stop=(kt == KT_D - 1) and not bias,
                )
            if bias:
                nc.tensor.matmul(
                    ps,
                    lhsT=bq_s[0:1, m * 128 : (m + 1) * 128],
                    rhs=ones_row[0:1, :],
                    start=False,
                    stop=True,
                )
            nc.vector.tensor_copy(dst[:, c * 512 : (c + 1) * 512], ps)
            if after is not None:
                after()

        def v_chunk(st):
            ps = qkp.tile([128, 512], F32, tag="proj", name=f"vproj{st}")
            pr, half = st // 2, st % 2
            for kt in range(KT_D):
                nc.tensor.matmul(
                    ps[:, 0:HD],
                    lhsT=xv_t[pr][:, kt, half * 128 : (half + 1) * 128],
                    rhs=wv_s[:, kt, :],
                    start=(kt == 0),
                    stop=(kt == KT_D - 1),
                )
            nc.vector.tensor_copy(vh_s[:, st, :], ps[:, 0:HD])
            # both halves of pair pr consumed -> its buffer may be reused
            if half == 1 and pr + 2 < NST // 2:
                load_xv(pr + 2)

        combos = [(qb, hp) for qb in range(NQB) for hp in range(2)]
        NC = len(combos)
        expT, ctx_ps = {}, {}

        def softmax_stats(ci):
            """DVE add-tree + GpSimd partition-sum + reciprocal, both heads."""
            et = expT[ci]
            with nc.allow_low_precision("bf16 softmax denominator tree"):
                for e in range(2):
                    s = 4 * e
                    sl = slice(s, s + 4)
                    nc.vector.tensor_add(t4[:, sl], et[:, 0:4, e, :], et[:, 4:8, e, :])
                    nc.vector.tensor_add(t4[:, sl], t4[:, sl], et[:, 8:12, e, :])
                    nc.vector.tensor_add(t4[:, sl], t4[:, sl], et[:, 12:16, e, :])
                    nc.vector.tensor_add(t4[:, s : s + 2], t4[:, s : s + 2], t4[:, s + 2 : s + 4])
                    nc.vector.tensor_add(t4[:, s : s + 1], t4[:, s : s + 1], t4[:, s + 1 : s + 2])
                for e in range(2):
                    nc.gpsimd.partition_all_reduce(
                        denf, t4[:, 4 * e, :], 128, bass_isa.ReduceOp.add
                    )
                    nc.vector.reciprocal_approx_fast(recipf, denf[0:1, :])
                    nc.vector.tensor_copy(recip[ci % 2][e], recipf)

        def bcast_mm(ci):
            bc = cb_pool.tile([128, 512], F32, tag="bcast", name=f"bc{ci}")
            for e in range(2):
                nc.tensor.matmul(
                    bc[e * 64 : (e + 1) * 64, :],
                    lhsT=ones_col[0:1, :],
                    rhs=recip[ci % 2][e][0:1, :],
                    start=True,
                    stop=True,
                    tile_position=(0, e * 64),
                )
            # DVE may read only one PSUM operand in the normalize multiply
            nc.vector.tensor_copy(bcs, bc)
            return bcs

        def normalize(ci, bc):
            qb, hp = combos[ci]
            cp = ctx_ps[ci]
            for e in range(2):
                sl = slice(e * 64, (e + 1) * 64)
                nc.vector.tensor_mul(
                    ctxT[hp][sl, qb * 512 : (qb + 1) * 512], cp[sl, e, :], bc[sl, :]
                )
            del expT[ci], ctx_ps[ci]

        def pv_step(ci, kt):
            qb, hp = combos[ci]
            if kt == 0:
                ctx_ps[ci] = cb_pool.tile([128, 2, 512], F32, tag="ctx", name=f"ctx{ci}")
            for e in range(2):
                nc.tensor.matmul(
                    ctx_ps[ci][e * 64 : (e + 1) * 64, e, :],
                    lhsT=vh_s[:, kt, (hp * 2 + e) * 64 : (hp * 2 + e + 1) * 64],
                    rhs=expT[ci][:, kt, e, :],
                    start=(kt == 0),
                    stop=(kt == NST - 1),
                    tile_position=(0, e * 64),
                )

        outp_pool = [None]

        def outproj_quantum(qb, stl):
            st = qb * 4 + stl
            outp = outp_pool[0]
            for nb in range(2):
                ps = outp.tile([128, 512], F32, tag="op", name=f"op{st}_{nb}")
                for kt2 in range(2):
                    nc.tensor.matmul(
                        ps,
                        lhsT=ctxT[kt2][:, st * 128 : (st + 1) * 128],
                        rhs=wo_s[:, kt2, nb * 512 : (nb + 1) * 512],
                        start=(kt2 == 0),
                        stop=(kt2 == 1),
                    )
                ob = ostage_pool.tile([128, 512], F32, tag="ob")
                nc.vector.tensor_copy(ob, ps)
                nc.gpsimd.dma_start(
                    out[st * 128 : (st + 1) * 128, nb * 512 : (nb + 1) * 512], ob
                )

        # ---------------- prologue ---------------------------------------
        # Chunk 0 of khT (both halves) + chunk 0 of qhT (both halves):
        # combo 0's first 4 score kt-tiles only need these.
        qk_chunk(khT[0], wk_s, xk_t, 0, 0, bias=False)
        qk_chunk(khT[1], wk_s, xk_t, 1, 0, bias=False, after=lambda: load_xk(2))
        qk_chunk(qhT[0], wq_s, xq_t, 0, 0, bias=True)
        qk_chunk(qhT[1], wq_s, xq_t, 1, 0, bias=True, after=lambda: load_xq(2))

        # Filler schedule: {(ci, kt): [thunk, ...]}.  K chunk c feeds score
        # kt-tiles 4c..4c+3, so chunk c is emitted well before step 4c.
        fill = {}

        def F(ci, kt, fn):
            fill.setdefault((ci, kt), []).append(fn)

        F(0, 0, lambda: qk_chunk(khT[0], wk_s, xk_t, 0, 1, bias=False))
        F(0, 1, lambda: qk_chunk(khT[1], wk_s, xk_t, 1, 1, bias=False,
                                 after=lambda: load_xk(3)))
        F(0, 4, lambda: qk_chunk(khT[0], wk_s, xk_t, 0, 2, bias=False))
        F(0, 5, lambda: qk_chunk(khT[1], wk_s, xk_t, 1, 2, bias=False))
        F(0, 8, lambda: qk_chunk(khT[0], wk_s, xk_t, 0, 3, bias=False))
        F(0, 9, lambda: qk_chunk(khT[1], wk_s, xk_t, 1, 3, bias=False))
        F(0, 12, lambda: qk_chunk(qhT[0], wq_s, xq_t, 0, 1, bias=True))
        F(0, 14, lambda: qk_chunk(qhT[1], wq_s, xq_t, 1, 1, bias=True,
                                  after=lambda: load_xq(3)))
        # V projection: st 0..7 during ci1, st 8..15 during ci2 (PV(0) at
        # combo 2 consumes st_kt at step kt; st8 lands at ci2 step 0).
        for i, st in enumerate(range(8)):
            F(1, 2 * i, lambda st=st: v_chunk(st))
        for i, st in enumerate(range(8, NST)):
            F(2, 2 * i, lambda st=st: v_chunk(st))
        # remaining q chunks
        F(1, 11, lambda: qk_chunk(qhT[0], wq_s, xq_t, 0, 2, bias=True))
        F(1, 13, lambda: qk_chunk(qhT[1], wq_s, xq_t, 1, 2, bias=True))
        F(2, 11, lambda: qk_chunk(qhT[0], wq_s, xq_t, 0, 3, bias=True))
        F(2, 13, lambda: qk_chunk(qhT[1], wq_s, xq_t, 1, 3, bias=True))
        # out-projection quanta: qb0 -> ci4, qb1 -> ci6 (qb2/qb3 in tail)
        for i in range(4):
            F(4, 2 * i + 5, lambda i=i: outproj_quantum(0, i))
            F(6, 2 * i + 5, lambda i=i: outproj_quantum(1, i))

        # ---------------- main pipeline ----------------------------------
        pend_bc = {}
        for ci, (qb, hp) in enumerate(combos):
            et = expT_pool.tile([128, NST, 2, 512], BF16, tag="expT", name=f"expT{ci}")
            expT[ci] = et
            sc = None
            for kt in range(NST):
                if kt % 2 == 0:
                    sc = sc_pool.tile([128, 2, 2, 512], F32, tag="sc", name=f"sc{ci}_{kt}")
                for e in range(2):
                    po = e * 64
                    nc.tensor.matmul(
                        sc[:, kt % 2, e, :],
                        lhsT=khT[hp][po : po + 64, kt * 128 : (kt + 1) * 128],
                        rhs=qhT[hp][po : po + 64, qb * 512 : (qb + 1) * 512],
                        start=True,
                        stop=True,
                        tile_position=(po, 0),
                    )
                if kt % 2 == 1:
                    nc.scalar.activation(et[:, kt - 1 : kt + 1, :, :], sc, AF.Exp)
                if ci >= LAG:
                    pv_step(ci - LAG, kt)
                for fn in fill.pop((ci, kt), ()):
                    fn()
                if kt == 1 and ci >= 1:
                    softmax_stats(ci - 1)
                if kt == 13 and ci >= LAG:
                    pend_bc[ci - LAG] = bcast_mm(ci - LAG)
                if ci == 3 and kt == 0:
                    # all projection work emitted; swap proj PSUM for out-proj
                    proj_stack.close()
                    outp_pool[0] = ctx.enter_context(
                        tc.tile_pool(name="out_psum", bufs=1, space="PSUM")
                    )
            if ci >= LAG:
                normalize(ci - LAG, pend_bc.pop(ci - LAG))

        # ---------------- tail -------------------------------------------
        for ci in (NC - 2, NC - 1):
            for kt in range(NST):
                pv_step(ci, kt)
                if ci == NC - 1 and kt in (5, 7, 9, 11):
                    outproj_quantum(2, (kt - 5) // 2)
            if ci == NC - 1:
                softmax_stats(ci)
            normalize(ci, bcast_mm(ci))
        for stl in range(4):
            outproj_quantum(3, stl)

    nc.compile()
    return nc


_CACHE: dict = {}


def _get_nc() -> bass.Bass:
    if "nc" not in _CACHE:
        _CACHE["nc"] = build_nc()
    return _CACHE["nc"]


def _bf16(x: np.ndarray) -> np.ndarray:
    return np.ascontiguousarray(x).astype(ml_dtypes.bfloat16)


def make_in_maps(q, k, v, wq, bq, wk, bk, wv, bv, wo, bo):
    scale = np.float32(1.0 / np.sqrt(DK))
    in_maps = []
    for c in range(NCORES):
        b, g = divmod(c, 4)
        hh = g * HD
        in_maps.append(
            {
                "xqT": _bf16(np.asarray(q[b], np.float32).T),
                "xkT": _bf16(np.asarray(k[b], np.float32).T),
                "xvT": _bf16(np.asarray(v[b], np.float32).T),
                "wqT": _bf16(np.asarray(wq[hh : hh + HD], np.float32).T * scale),
                "wkT": _bf16(np.asarray(wk[hh : hh + HD], np.float32).T),
                "wvT": _bf16(np.asarray(wv[hh : hh + HD], np.float32).T),
                "woT": _bf16(np.asarray(wo[:, hh : hh + HD], np.float32).T),
                "bq": _bf16(np.asarray(bq[hh : hh + HD], np.float32) * scale),
            }
        )
    return in_maps


def assemble(results, bv, bo, wo) -> np.ndarray:
    out = np.zeros((B, S, D), np.float32)
    for c in range(NCORES):
        out[c // 4] += np.asarray(results[c]["out_partial"], np.float32)
    corr = np.asarray(bv, np.float32) @ np.asarray(wo, np.float32).T + np.asarray(
        bo, np.float32
    )
    out += corr[None, None, :]
    return out


def kernel(q, k, v, wq, bq, wk, bk, wv, bv, wo, bo) -> np.ndarray:
    nc = _get_nc()
    in_maps = make_in_maps(q, k, v, wq, bq, wk, bk, wv, bv, wo, bo)
    res = run_bass_kernel_spmd(nc, in_maps, list(range(NCORES))).results
    return assemble(res, bv, bo, wo)


# revision 15
# speedup vs baseline: 1.4400x; 1.1282x over previous
"""Multi-head attention (B=2, S=2048, D=1024, H=16, dk=64) on 8 trn2 cores.

Sharding: core c handles batch b=c//4 and 4 heads g=c%4 (heads 4g..4g+3).
Each core computes its heads' Q/K/V projections, attention, and a partial
output projection; the host sums the 4 partials per batch.

v3 — transposed-scores layout (k on partitions), denominator fused into PV:
  - qhT/khT [128 = 2 heads x 64 dk, S] bf16; vh [128 k, 16 kt, 4 h, 65]
    bf16 with a ones column at index 64.
  - scoresT [128 k, 512 q] per (head, kt): lhsT = khT slice (K=64 rows at
    partition base po), rhs = qhT slice. The two heads of a pair are
    row-packed (tile_position (0,0)/(64,0)) and run concurrently on the PE.
  - exp on ScalarE from PSUM [128, 2, 2, 512] (one kt pair, both heads) to
    bf16 expT in SBUF. No max subtraction (scores ~ N(0,1)).
  - PV contracts k on partitions: lhsT = [vh | ones] [128, 65], rhs = expT
    slice; head e accumulates into its own ctx PSUM bank [0:65, e, :] over
    the 16 kt. Row 64 is the softmax denominator — computed by the PE for
    free, in fp32, consistent with the numerator.
  - per combo: reciprocal_approx_fast on the denominator row, cast to bf16,
    partition-broadcast via K=1 PE matmuls (ones column) into an out-proj
    PSUM tile, and the normalize-multiply is fused into the ctx PSUM->SBUF
    copy on DVE.
  - out projection reads ctxT (already [hd, q]) against resident wo.
  - bk/bv/bo never touch the device: bk shifts scores by a per-row constant
    (softmax-invariant), bv/bo fold into a host-side correction, bq is a
    rank-1 matmul on-device (x 1/sqrt(dk), folded into wq/bq on the host).

Emission is software-pipelined so ScalarE (~126us of exp) streams: combo ci
emits scores(ci), PV(ci-2) offset by 3 kt steps (so the normalize chain of
the previous ctx tile never stalls the PE), and filler quanta (projection
chunks / out-proj quanta). PSUM: sc(4 banks) + ctx(2) + proj(2, bufs=2) = 8;
the out-proj/broadcast pool (2) replaces the proj pool at combo 2.
"""

import sys

for _p in ("/opt/trn_rl_repo",):
    if _p not in sys.path:
        sys.path.insert(0, _p)

from contextlib import ExitStack

import ml_dtypes
import numpy as np

import concourse.bass as bass
import concourse.bacc as bacc_mod
import concourse.mybir as mybir
import concourse.tile as tile
from concourse.bass_utils import run_bass_kernel_spmd

BF16 = mybir.dt.bfloat16
F32 = mybir.dt.float32
AF = mybir.ActivationFunctionType

B, S, D = 2, 2048, 1024
NCORES = 8
HLOC = 4          # heads per core
DK = 64
HD = HLOC * DK    # local head dims = 256
KT_D = D // 128   # 8 k-tiles over the model dim
NST = S // 128    # 16 tiles over sequence (kv)
NQB = 4           # q blocks of 512
NCH = 4           # proj output chunks of 512 over S
LAG = 2           # PV trails scores by LAG combos
PV_OFF = 3        # PV step kt runs at parent step kt+PV_OFF


def build_nc() -> bass.Bass:
    nc = bacc_mod.Bacc()

    xqT = nc.dram_tensor("xqT", [D, S], BF16, kind="ExternalInput")
    xkT = nc.dram_tensor("xkT", [D, S], BF16, kind="ExternalInput")
    xvT = nc.dram_tensor("xvT", [D, S], BF16, kind="ExternalInput")
    wqT = nc.dram_tensor("wqT", [D, HD], BF16, kind="ExternalInput")
    wkT = nc.dram_tensor("wkT", [D, HD], BF16, kind="ExternalInput")
    wvT = nc.dram_tensor("wvT", [D, HD], BF16, kind="ExternalInput")
    woT = nc.dram_tensor("woT", [HD, D], BF16, kind="ExternalInput")
    bqd = nc.dram_tensor("bq", [HD], BF16, kind="ExternalInput")
    out = nc.dram_tensor("out_partial", [S, D], F32, kind="ExternalOutput")

    with tile.TileContext(nc) as tc, ExitStack() as ctx:
        const = ctx.enter_context(tc.tile_pool(name="const", bufs=1))
        persist = ctx.enter_context(tc.tile_pool(name="persist", bufs=1))

        wq_s = const.tile([128, KT_D, HD], BF16, tag="wq")
        wk_s = const.tile([128, KT_D, HD], BF16, tag="wk")
        wv_s = const.tile([128, KT_D, HD], BF16, tag="wv")
        wo_s = const.tile([128, 2, D], BF16, tag="wo")
        bq_s = const.tile([1, HD], BF16, tag="bq")
        ones_row = const.tile([1, 512], BF16, tag="ones")
        ones_col = const.tile([1, DK], BF16, tag="onesc")
        nc.vector.memset(ones_row, 1.0)
        nc.vector.memset(ones_col, 1.0)

        # sync queue: wk, wq, bq, xq chunks..., wv, wo. gpsimd queue: xk
        # chunks, xv chunks, output stores.
        nc.sync.dma_start(wk_s, wkT.rearrange("(t p) n -> p t n", p=128))
        nc.sync.dma_start(wq_s, wqT.rearrange("(t p) n -> p t n", p=128))
        nc.sync.dma_start(bq_s, bqd.rearrange("(o n) -> o n", o=1))

        qhT = [persist.tile([128, S], BF16, tag=f"qhT{m}", name=f"qhT{m}") for m in range(2)]
        khT = [persist.tile([128, S], BF16, tag=f"khT{m}", name=f"khT{m}") for m in range(2)]
        # vh with a ones column per (kt, head): PV computes the softmax
        # denominator into ctx row 64 for free.
        vh_s = persist.tile([128, NST, HLOC, DK + 1], BF16, tag="vh")
        nc.vector.memset(vh_s[:, :, :, DK], 1.0)
        ctxT = [persist.tile([128, S], BF16, tag=f"ctxT{m}", name=f"ctxT{m}") for m in range(2)]

        xkp = ctx.enter_context(tc.tile_pool(name="xkload", bufs=2))
        xqp = ctx.enter_context(tc.tile_pool(name="xqload", bufs=2))
        xvp = ctx.enter_context(tc.tile_pool(name="xvload", bufs=2))
        xk_t, xq_t, xv_t = {}, {}, {}

        def load_xk(c):
            t = xkp.tile([128, KT_D, 512], BF16, tag="xk", name=f"xk{c}")
            nc.gpsimd.dma_start(
                t, xkT[:, c * 512 : (c + 1) * 512].rearrange("(t p) n -> p t n", p=128)
            )
            xk_t[c] = t

        def load_xq(c):
            t = xqp.tile([128, KT_D, 512], BF16, tag="xq", name=f"xq{c}")
            nc.sync.dma_start(
                t, xqT[:, c * 512 : (c + 1) * 512].rearrange("(t p) n -> p t n", p=128)
            )
            xq_t[c] = t

        def load_xv(pr):
            t = xvp.tile([128, KT_D, 256], BF16, tag="xv", name=f"xv{pr}")
            nc.gpsimd.dma_start(
                t, xvT[:, pr * 256 : (pr + 1) * 256].rearrange("(t p) n -> p t n", p=128)
            )
            xv_t[pr] = t

        load_xk(0)
        load_xk(1)
        load_xq(0)
        load_xq(1)
        load_xv(0)
        load_xv(1)
        nc.sync.dma_start(wv_s, wvT.rearrange("(t p) n -> p t n", p=128))
        nc.sync.dma_start(wo_s, woT.rearrange("(t p) n -> p t n", p=128))

        # softmax stat tiles
        denf = persist.tile([1, 2, 512], F32, tag="denf")
        recipf = persist.tile([1, 512], F32, tag="recipf")
        recip = [persist.tile([1, 512], BF16, tag=f"rec{e}", name=f"rec{e}") for e in range(2)]
        bcs = persist.tile([128, 512], BF16, tag="bcs")

        expT_pool = ctx.enter_context(tc.tile_pool(name="expT", bufs=LAG + 1))
        ostage_pool = ctx.enter_context(tc.tile_pool(name="ostage", bufs=2))

        sc_pool = ctx.enter_context(tc.tile_pool(name="sc_psum", bufs=1, space="PSUM"))
        cb_pool = ctx.enter_context(tc.tile_pool(name="cb_psum", bufs=1, space="PSUM"))

        proj_stack = ExitStack()
        qkp = proj_stack.enter_context(tc.tile_pool(name="qk_psum", bufs=2, space="PSUM"))

        # ---------------- emission helpers -------------------------------
        def qk_chunk(dst, w_s, xt, m, c, bias, after=None):
            ps = qkp.tile([128, 512], F32, tag="proj")
            for kt in range(KT_D):
                nc.tensor.matmul(
                    ps,
                    lhsT=w_s[:, kt, m * 128 : (m + 1) * 128],
                    rhs=xt[c][:, kt, :],
                    start=(kt == 0),
                    stop=(kt == KT_D - 1) and not bias,
                )
            if bias:
                nc.tensor.matmul(
                    ps,
                    lhsT=bq_s[0:1, m * 128 : (m + 1) * 128],
                    rhs=ones_row[0:1, :],
                    start=False,
                    stop=True,
                )
            nc.vector.tensor_copy(dst[:, c * 512 : (c + 1) * 512], ps)
            if after is not None:
                after()

        def v_chunk(st):
            ps = qkp.tile([128, 512], F32, tag="proj", name=f"vproj{st}")
            pr, half = st // 2, st % 2
            for kt in range(KT_D):
                nc.tensor.matmul(
                    ps[:, 0:HD],
                    lhsT=xv_t[pr][:, kt, half * 128 : (half + 1) * 128],
                    rhs=wv_s[:, kt, :],
                    start=(kt == 0),
                    stop=(kt == KT_D - 1),
                )
            nc.vector.tensor_copy(
                vh_s[:, st, :, 0:DK],
                ps[:, 0:HD].rearrange("p (h d) -> p h d", h=HLOC),
            )
            if half == 1 and pr + 2 < NST // 2:
                load_xv(pr + 2)

        combos = [(qb, hp) for qb in range(NQB) for hp in range(2)]
        NC = len(combos)
        expT, ctx_ps = {}, {}

        def stats(ci):
            """1/denominator from ctx row 64, per head (after PV(ci))."""
            for e in range(2):
                # stage the PSUM denominator row through SBUF: the custom-DVE
                # reciprocal misreads PSUM for some values on HW
                nc.vector.tensor_copy(denf[:, e, :], ctx_ps[ci][64:65, e, :])
            for e in range(2):
                nc.vector.reciprocal_approx_fast(recipf, denf[:, e, :])
                nc.vector.tensor_copy(recip[e], recipf)

        def bcast_mm(ci):
            bc = outp_pool[0].tile([128, 512], F32, tag="op", name=f"bc{ci}")
            for e in range(2):
                nc.tensor.matmul(
                    bc[e * 64 : (e + 1) * 64, :],
                    lhsT=ones_col[0:1, :],
                    rhs=recip[e][0:1, :],
                    start=True,
                    stop=True,
                    tile_position=(0, e * 64),
                )
            nc.vector.tensor_copy(bcs, bc)

        def normalize(ci):
            qb, hp = combos[ci]
            cp = ctx_ps[ci]
            for e in range(2):
                sl = slice(e * 64, (e + 1) * 64)
                nc.vector.tensor_mul(
                    ctxT[hp][sl, qb * 512 : (qb + 1) * 512], cp[0:64, e, :], bcs[sl, :]
                )
            del expT[ci], ctx_ps[ci]

        def pv_step(ci, kt):
            qb, hp = combos[ci]
            if kt == 0:
                ctx_ps[ci] = cb_pool.tile([128, 2, 512], F32, tag="ctx", name=f"ctx{ci}")
            for e in range(2):
                nc.tensor.matmul(
                    ctx_ps[ci][0 : DK + 1, e, :],
                    lhsT=vh_s[:, kt, hp * 2 + e, :],
                    rhs=expT[ci][:, kt, e, :],
                    start=(kt == 0),
                    stop=(kt == NST - 1),
                )

        outp_pool = [None]

        def outproj_quantum(qb, stl):
            st = qb * 4 + stl
            outp = outp_pool[0]
            for nb in range(2):
                ps = outp.tile([128, 512], F32, tag="op", name=f"op{st}_{nb}")
                for kt2 in range(2):
                    nc.tensor.matmul(
                        ps,
                        lhsT=ctxT[kt2][:, st * 128 : (st + 1) * 128],
                        rhs=wo_s[:, kt2, nb * 512 : (nb + 1) * 512],
                        start=(kt2 == 0),
                        stop=(kt2 == 1),
                    )
                ob = ostage_pool.tile([128, 512], F32, tag="ob")
                nc.vector.tensor_copy(ob, ps)
                nc.gpsimd.dma_start(
                    out[st * 128 : (st + 1) * 128, nb * 512 : (nb + 1) * 512], ob
                )

        # ---------------- prologue ---------------------------------------
        # Chunk 0 of khT (both halves) + chunk 0 of qhT (both halves):
        # combo 0's first 4 score kt-tiles only need these.
        qk_chunk(khT[0], wk_s, xk_t, 0, 0, bias=False)
        qk_chunk(khT[1], wk_s, xk_t, 1, 0, bias=False, after=lambda: load_xk(2))
        qk_chunk(qhT[0], wq_s, xq_t, 0, 0, bias=True)
        qk_chunk(qhT[1], wq_s, xq_t, 1, 0, bias=True, after=lambda: load_xq(2))

        # Filler schedule: {(ci, kt): [thunk, ...]}.  K chunk c feeds score
        # kt-tiles 4c..4c+3, so chunk c is emitted well before step 4c.
        fill = {}

        def F(ci, kt, fn):
            fill.setdefault((ci, kt), []).append(fn)

        F(0, 0, lambda: qk_chunk(khT[0], wk_s, xk_t, 0, 1, bias=False))
        F(0, 1, lambda: qk_chunk(khT[1], wk_s, xk_t, 1, 1, bias=False,
                                 after=lambda: load_xk(3)))
        F(0, 4, lambda: qk_chunk(khT[0], wk_s, xk_t, 0, 2, bias=False))
        F(0, 5, lambda: qk_chunk(khT[1], wk_s, xk_t, 1, 2, bias=False))
        F(0, 8, lambda: qk_chunk(khT[0], wk_s, xk_t, 0, 3, bias=False))
        F(0, 9, lambda: qk_chunk(khT[1], wk_s, xk_t, 1, 3, bias=False))
        F(0, 12, lambda: qk_chunk(qhT[0], wq_s, xq_t, 0, 1, bias=True))
        F(0, 14, lambda: qk_chunk(qhT[1], wq_s, xq_t, 1, 1, bias=True,
                                  after=lambda: load_xq(3)))
        # V projection: st 0..7 during ci1, st 8..15 during ci2 (PV(0) at
        # combo 2 consumes st_kt at step kt+PV_OFF).
        for i, st in enumerate(range(8)):
            F(1, 2 * i, lambda st=st: v_chunk(st))
        for i, st in enumerate(range(8, NST)):
            F(2, 2 * i, lambda st=st: v_chunk(st))
        # remaining q chunks (c2 late in ci1, c3 early in ci2 so the proj
        # pool can close at ci2 kt6)
        F(1, 11, lambda: qk_chunk(qhT[0], wq_s, xq_t, 0, 2, bias=True))
        F(1, 13, lambda: qk_chunk(qhT[1], wq_s, xq_t, 1, 2, bias=True))
        F(2, 1, lambda: qk_chunk(qhT[0], wq_s, xq_t, 0, 3, bias=True))
        F(2, 3, lambda: qk_chunk(qhT[1], wq_s, xq_t, 1, 3, bias=True))
        # out-projection quanta: qb0 -> ci4, qb1 -> ci6 (qb2/qb3 in tail)
        for i in range(4):
            F(4, 2 * i + 5, lambda i=i: outproj_quantum(0, i))
            F(6, 2 * i + 5, lambda i=i: outproj_quantum(1, i))

        # ---------------- main pipeline ----------------------------------
        for ci, (qb, hp) in enumerate(combos):
            et = expT_pool.tile([128, NST, 2, 512], BF16, tag="expT", name=f"expT{ci}")
            expT[ci] = et
            sc = None
            for kt in range(NST):
                if kt % 2 == 0:
                    sc = sc_pool.tile([128, 2, 2, 512], F32, tag="sc", name=f"sc{ci}_{kt}")
                for e in range(2):
                    po = e * 64
                    nc.tensor.matmul(
                        sc[:, kt % 2, e, :],
                        lhsT=khT[hp][po : po + 64, kt * 128 : (kt + 1) * 128],
                        rhs=qhT[hp][po : po + 64, qb * 512 : (qb + 1) * 512],
                        start=True,
                        stop=True,
                        tile_position=(po, 0),
                    )
                if kt % 2 == 1:
                    nc.scalar.activation(et[:, kt - 1 : kt + 1, :, :], sc, AF.Exp)
                if kt == 1 and ci >= LAG + 1:
                    bcast_mm(ci - LAG - 1)
                if kt == 2 and ci >= LAG + 1:
                    normalize(ci - LAG - 1)
                if kt >= PV_OFF and ci >= LAG:
                    pv_step(ci - LAG, kt - PV_OFF)
                for fn in fill.pop((ci, kt), ()):
                    fn()
                if ci == 2 and kt == 15:
                    # all projection work emitted; swap proj PSUM for
                    # out-proj + broadcast
                    proj_stack.close()
                    outp_pool[0] = ctx.enter_context(
                        tc.tile_pool(name="out_psum", bufs=2, space="PSUM")
                    )
            if ci >= LAG:
                for kt in range(NST - PV_OFF, NST):
                    pv_step(ci - LAG, kt)
                stats(ci - LAG)

        # ---------------- tail -------------------------------------------
        bcast_mm(NC - 3)
        normalize(NC - 3)
        for ci in (NC - 2, NC - 1):
            for kt in range(NST):
                pv_step(ci, kt)
                if ci == NC - 2 and kt in (5, 7, 9, 11):
                    outproj_quantum(2, (kt - 5) // 2)
            stats(ci)
            bcast_mm(ci)
            normalize(ci)
        for stl in range(4):
            outproj_quantum(3, stl)

    nc.compile()
    return nc


_CACHE: dict = {}


def _get_nc() -> bass.Bass:
    if "nc" not in _CACHE:
        _CACHE["nc"] = build_nc()
    return _CACHE["nc"]


def _bf16(x: np.ndarray) -> np.ndarray:
    return np.ascontiguousarray(x).astype(ml_dtypes.bfloat16)


def make_in_maps(q, k, v, wq, bq, wk, bk, wv, bv, wo, bo):
    scale = np.float32(1.0 / np.sqrt(DK))
    in_maps = []
    for c in range(NCORES):
        b, g = divmod(c, 4)
        hh = g * HD
        in_maps.append(
            {
                "xqT": _bf16(np.asarray(q[b], np.float32).T),
                "xkT": _bf16(np.asarray(k[b], np.float32).T),
                "xvT": _bf16(np.asarray(v[b], np.float32).T),
                "wqT": _bf16(np.asarray(wq[hh : hh + HD], np.float32).T * scale),
                "wkT": _bf16(np.asarray(wk[hh : hh + HD], np.float32).T),
                "wvT": _bf16(np.asarray(wv[hh : hh + HD], np.float32).T),
                "woT": _bf16(np.asarray(wo[:, hh : hh + HD], np.float32).T),
                "bq": _bf16(np.asarray(bq[hh : hh + HD], np.float32) * scale),
            }
        )
    return in_maps


def assemble(results, bv, bo, wo) -> np.ndarray:
    out = np.zeros((B, S, D), np.float32)
    for c in range(NCORES):
        out[c // 4] += np.asarray(results[c]["out_partial"], np.float32)
    corr = np.asarray(bv, np.float32) @ np.asarray(wo, np.float32).T + np.asarray(
        bo, np.float32
    )
    out += corr[None, None, :]
    return out


def kernel(q, k, v, wq, bq, wk, bk, wv, bv, wo, bo) -> np.ndarray:
    nc = _get_nc()
    in_maps = make_in_maps(q, k, v, wq, bq, wk, bk, wv, bv, wo, bo)
    res = run_bass_kernel_spmd(nc, in_maps, list(range(NCORES))).results
    return assemble(res, bv, bo, wo)


# revision 16
# speedup vs baseline: 1.4441x; 1.0028x over previous
"""Multi-head attention (B=2, S=2048, D=1024, H=16, dk=64) on 8 trn2 cores.

Sharding: core c handles batch b=c//4 and 4 heads g=c%4 (heads 4g..4g+3).
Each core computes its heads' Q/K/V projections, attention, and a partial
output projection; the host sums the 4 partials per batch.

v3 — transposed-scores layout (k on partitions), denominator fused into PV:
  - qhT/khT [128 = 2 heads x 64 dk, S] bf16; vh [128 k, 16 kt, 4 h, 65]
    bf16 with a ones column at index 64.
  - scoresT [128 k, 512 q] per (head, kt): lhsT = khT slice (K=64 rows at
    partition base po), rhs = qhT slice. The two heads of a pair are
    row-packed (tile_position (0,0)/(64,0)) and run concurrently on the PE.
  - exp on ScalarE from PSUM [128, 2, 2, 512] (one kt pair, both heads) to
    bf16 expT in SBUF. No max subtraction (scores ~ N(0,1)).
  - PV contracts k on partitions: lhsT = [vh | ones] [128, 65], rhs = expT
    slice; head e accumulates into its own ctx PSUM bank [0:65, e, :] over
    the 16 kt. Row 64 is the softmax denominator — computed by the PE for
    free, in fp32, consistent with the numerator.
  - per combo: reciprocal_approx_fast on the denominator row, cast to bf16,
    partition-broadcast via K=1 PE matmuls (ones column) into an out-proj
    PSUM tile, and the normalize-multiply is fused into the ctx PSUM->SBUF
    copy on DVE.
  - out projection reads ctxT (already [hd, q]) against resident wo.
  - bk/bv/bo never touch the device: bk shifts scores by a per-row constant
    (softmax-invariant), bv/bo fold into a host-side correction, bq is a
    rank-1 matmul on-device (x 1/sqrt(dk), folded into wq/bq on the host).

Emission is software-pipelined so ScalarE (~126us of exp) streams: combo ci
emits scores(ci), PV(ci-2) offset by 3 kt steps (so the normalize chain of
the previous ctx tile never stalls the PE), and filler quanta (projection
chunks / out-proj quanta). PSUM: sc(4 banks) + ctx(2) + proj(2, bufs=2) = 8;
the out-proj/broadcast pool (2) replaces the proj pool at combo 2.
"""

import sys

for _p in ("/opt/trn_rl_repo",):
    if _p not in sys.path:
        sys.path.insert(0, _p)

from contextlib import ExitStack

import ml_dtypes
import numpy as np

import concourse.bass as bass
import concourse.bacc as bacc_mod
import concourse.mybir as mybir
import concourse.tile as tile
from concourse.bass_utils import run_bass_kernel_spmd

BF16 = mybir.dt.bfloat16
F32 = mybir.dt.float32
AF = mybir.ActivationFunctionType

B, S, D = 2, 2048, 1024
NCORES = 8
HLOC = 4          # heads per core
DK = 64
HD = HLOC * DK    # local head dims = 256
KT_D = D // 128   # 8 k-tiles over the model dim
NST = S // 128    # 16 tiles over sequence (kv)
NQB = 4           # q blocks of 512
NCH = 4           # proj output chunks of 512 over S
LAG = 2           # PV trails scores by LAG combos
PV_OFF = 3        # PV step kt runs at parent step kt+PV_OFF


def build_nc() -> bass.Bass:
    nc = bacc_mod.Bacc()

    xqT = nc.dram_tensor("xqT", [D, S], BF16, kind="ExternalInput")
    xkT = nc.dram_tensor("xkT", [D, S], BF16, kind="ExternalInput")
    xvT = nc.dram_tensor("xvT", [D, S], BF16, kind="ExternalInput")
    wqT = nc.dram_tensor("wqT", [D, HD], BF16, kind="ExternalInput")
    wkT = nc.dram_tensor("wkT", [D, HD], BF16, kind="ExternalInput")
    wvT = nc.dram_tensor("wvT", [D, HD], BF16, kind="ExternalInput")
    woT = nc.dram_tensor("woT", [HD, D], BF16, kind="ExternalInput")
    bqd = nc.dram_tensor("bq", [HD], BF16, kind="ExternalInput")
    out = nc.dram_tensor("out_partial", [S, D], F32, kind="ExternalOutput")

    with tile.TileContext(nc) as tc, ExitStack() as ctx:
        const = ctx.enter_context(tc.tile_pool(name="const", bufs=1))
        persist = ctx.enter_context(tc.tile_pool(name="persist", bufs=1))

        wq_s = const.tile([128, KT_D, HD], BF16, tag="wq")
        wk_s = const.tile([128, KT_D, HD], BF16, tag="wk")
        wv_s = const.tile([128, KT_D, HD], BF16, tag="wv")
        wo_s = const.tile([128, 2, D], BF16, tag="wo")
        bq_s = const.tile([1, HD], BF16, tag="bq")
        ones_row = const.tile([1, 512], BF16, tag="ones")
        ones_col = const.tile([1, DK], BF16, tag="onesc")
        nc.vector.memset(ones_row, 1.0)
        nc.vector.memset(ones_col, 1.0)

        # sync queue: wk, wq, bq, xq chunks..., wv, wo. gpsimd queue: xk
        # chunks, xv chunks, output stores.
        nc.sync.dma_start(wk_s, wkT.rearrange("(t p) n -> p t n", p=128))
        nc.sync.dma_start(wq_s, wqT.rearrange("(t p) n -> p t n", p=128))
        nc.sync.dma_start(bq_s, bqd.rearrange("(o n) -> o n", o=1))

        qhT = [persist.tile([128, S], BF16, tag=f"qhT{m}", name=f"qhT{m}") for m in range(2)]
        khT = [persist.tile([128, S], BF16, tag=f"khT{m}", name=f"khT{m}") for m in range(2)]
        # vh with a ones column per (kt, head): PV computes the softmax
        # denominator into ctx row 64 for free.
        vh_s = persist.tile([128, NST, HLOC, DK + 1], BF16, tag="vh")
        nc.vector.memset(vh_s[:, :, :, DK], 1.0)
        ctxT = [persist.tile([128, S], BF16, tag=f"ctxT{m}", name=f"ctxT{m}") for m in range(2)]

        xkp = ctx.enter_context(tc.tile_pool(name="xkload", bufs=2))
        xqp = ctx.enter_context(tc.tile_pool(name="xqload", bufs=2))
        xvp = ctx.enter_context(tc.tile_pool(name="xvload", bufs=2))
        xk_t, xq_t, xv_t = {}, {}, {}

        def load_xk(c):
            t = xkp.tile([128, KT_D, 512], BF16, tag="xk", name=f"xk{c}")
            nc.gpsimd.dma_start(
                t, xkT[:, c * 512 : (c + 1) * 512].rearrange("(t p) n -> p t n", p=128)
            )
            xk_t[c] = t

        def load_xq(c):
            t = xqp.tile([128, KT_D, 512], BF16, tag="xq", name=f"xq{c}")
            nc.sync.dma_start(
                t, xqT[:, c * 512 : (c + 1) * 512].rearrange("(t p) n -> p t n", p=128)
            )
            xq_t[c] = t

        def load_xv(pr):
            t = xvp.tile([128, KT_D, 256], BF16, tag="xv", name=f"xv{pr}")
            nc.gpsimd.dma_start(
                t, xvT[:, pr * 256 : (pr + 1) * 256].rearrange("(t p) n -> p t n", p=128)
            )
            xv_t[pr] = t

        load_xk(0)
        load_xk(1)
        load_xq(0)
        load_xq(1)
        load_xv(0)
        load_xv(1)
        nc.sync.dma_start(wv_s, wvT.rearrange("(t p) n -> p t n", p=128))
        nc.sync.dma_start(wo_s, woT.rearrange("(t p) n -> p t n", p=128))

        # softmax stat tiles
        denf = persist.tile([1, 2, 512], F32, tag="denf")
        recipf = persist.tile([1, 512], F32, tag="recipf")
        recip = [persist.tile([1, 512], BF16, tag=f"rec{e}", name=f"rec{e}") for e in range(2)]
        bcs = persist.tile([128, 512], BF16, tag="bcs")

        expT_pool = ctx.enter_context(tc.tile_pool(name="expT", bufs=LAG + 1))
        ostage_pool = ctx.enter_context(tc.tile_pool(name="ostage", bufs=2))

        sc_pool = ctx.enter_context(tc.tile_pool(name="sc_psum", bufs=2, space="PSUM"))
        cb_pool = ctx.enter_context(tc.tile_pool(name="cb_psum", bufs=1, space="PSUM"))

        proj_stack = ExitStack()
        qkp = proj_stack.enter_context(tc.tile_pool(name="qk_psum", bufs=2, space="PSUM"))

        # ---------------- emission helpers -------------------------------
        def qk_chunk(dst, w_s, xt, m, c, bias, after=None):
            ps = qkp.tile([128, 512], F32, tag="proj")
            for kt in range(KT_D):
                nc.tensor.matmul(
                    ps,
                    lhsT=w_s[:, kt, m * 128 : (m + 1) * 128],
                    rhs=xt[c][:, kt, :],
                    start=(kt == 0),
                    stop=(kt == KT_D - 1) and not bias,
                )
            if bias:
                nc.tensor.matmul(
                    ps,
                    lhsT=bq_s[0:1, m * 128 : (m + 1) * 128],
                    rhs=ones_row[0:1, :],
                    start=False,
                    stop=True,
                )
            nc.vector.tensor_copy(dst[:, c * 512 : (c + 1) * 512], ps)
            if after is not None:
                after()

        def v_chunk(st):
            ps = qkp.tile([128, 512], F32, tag="proj", name=f"vproj{st}")
            pr, half = st // 2, st % 2
            for kt in range(KT_D):
                nc.tensor.matmul(
                    ps[:, 0:HD],
                    lhsT=xv_t[pr][:, kt, half * 128 : (half + 1) * 128],
                    rhs=wv_s[:, kt, :],
                    start=(kt == 0),
                    stop=(kt == KT_D - 1),
                )
            nc.vector.tensor_copy(
                vh_s[:, st, :, 0:DK],
                ps[:, 0:HD].rearrange("p (h d) -> p h d", h=HLOC),
            )
            if half == 1 and pr + 2 < NST // 2:
                load_xv(pr + 2)

        combos = [(qb, hp) for qb in range(NQB) for hp in range(2)]
        NC = len(combos)
        expT, ctx_ps = {}, {}

        def stats(ci):
            """1/denominator from ctx row 64, per head (after PV(ci))."""
            for e in range(2):
                # stage the PSUM denominator row through SBUF: the custom-DVE
                # reciprocal misreads PSUM for some values on HW
                nc.vector.tensor_copy(denf[:, e, :], ctx_ps[ci][64:65, e, :])
            for e in range(2):
                nc.vector.reciprocal_approx_fast(recipf, denf[:, e, :])
                nc.vector.tensor_copy(recip[e], recipf)

        def bcast_mm(ci):
            bc = outp_pool[0].tile([128, 512], F32, tag="op", name=f"bc{ci}")
            for e in range(2):
                nc.tensor.matmul(
                    bc[e * 64 : (e + 1) * 64, :],
                    lhsT=ones_col[0:1, :],
                    rhs=recip[e][0:1, :],
                    start=True,
                    stop=True,
                    tile_position=(0, e * 64),
                )
            nc.vector.tensor_copy(bcs, bc)

        def normalize(ci):
            qb, hp = combos[ci]
            cp = ctx_ps[ci]
            for e in range(2):
                sl = slice(e * 64, (e + 1) * 64)
                nc.vector.tensor_mul(
                    ctxT[hp][sl, qb * 512 : (qb + 1) * 512], cp[0:64, e, :], bcs[sl, :]
                )
            del expT[ci], ctx_ps[ci]

        def pv_step(ci, kt):
            qb, hp = combos[ci]
            if kt == 0:
                ctx_ps[ci] = cb_pool.tile([128, 2, 512], F32, tag="ctx", name=f"ctx{ci}")
            for e in range(2):
                nc.tensor.matmul(
                    ctx_ps[ci][0 : DK + 1, e, :],
                    lhsT=vh_s[:, kt, hp * 2 + e, :],
                    rhs=expT[ci][:, kt, e, :],
                    start=(kt == 0),
                    stop=(kt == NST - 1),
                )

        outp_pool = [None]

        def outproj_quantum(qb, stl):
            st = qb * 4 + stl
            outp = outp_pool[0]
            for nb in range(2):
                ps = outp.tile([128, 512], F32, tag="op", name=f"op{st}_{nb}")
                for kt2 in range(2):
                    nc.tensor.matmul(
                        ps,
                        lhsT=ctxT[kt2][:, st * 128 : (st + 1) * 128],
                        rhs=wo_s[:, kt2, nb * 512 : (nb + 1) * 512],
                        start=(kt2 == 0),
                        stop=(kt2 == 1),
                    )
                ob = ostage_pool.tile([128, 512], F32, tag="ob")
                nc.vector.tensor_copy(ob, ps)
                nc.gpsimd.dma_start(
                    out[st * 128 : (st + 1) * 128, nb * 512 : (nb + 1) * 512], ob
                )

        # ---------------- prologue ---------------------------------------
        # Chunk 0 of khT (both halves) + chunk 0 of qhT (both halves):
        # combo 0's first 4 score kt-tiles only need these.
        qk_chunk(khT[0], wk_s, xk_t, 0, 0, bias=False)
        qk_chunk(qhT[0], wq_s, xq_t, 0, 0, bias=True)

        # Filler schedule: {(ci, kt): [thunk, ...]}.  K chunk c feeds score
        # kt-tiles 4c..4c+3, so chunk c is emitted well before step 4c.
        fill = {}

        def F(ci, kt, fn):
            fill.setdefault((ci, kt), []).append(fn)

        F(0, 0, lambda: qk_chunk(khT[1], wk_s, xk_t, 1, 0, bias=False,
                                 after=lambda: load_xk(2)))
        F(0, 1, lambda: qk_chunk(qhT[1], wq_s, xq_t, 1, 0, bias=True,
                                 after=lambda: load_xq(2)))
        F(0, 2, lambda: qk_chunk(khT[0], wk_s, xk_t, 0, 1, bias=False))
        F(0, 3, lambda: qk_chunk(khT[1], wk_s, xk_t, 1, 1, bias=False,
                                 after=lambda: load_xk(3)))
        F(0, 4, lambda: qk_chunk(khT[0], wk_s, xk_t, 0, 2, bias=False))
        F(0, 5, lambda: qk_chunk(khT[1], wk_s, xk_t, 1, 2, bias=False))
        F(0, 8, lambda: qk_chunk(khT[0], wk_s, xk_t, 0, 3, bias=False))
        F(0, 9, lambda: qk_chunk(khT[1], wk_s, xk_t, 1, 3, bias=False))
        F(0, 12, lambda: qk_chunk(qhT[0], wq_s, xq_t, 0, 1, bias=True))
        F(0, 14, lambda: qk_chunk(qhT[1], wq_s, xq_t, 1, 1, bias=True,
                                  after=lambda: load_xq(3)))
        # V projection: st 0..7 during ci1, st 8..15 during ci2 (PV(0) at
        # combo 2 consumes st_kt at step kt+PV_OFF).
        for i, st in enumerate(range(8)):
            F(1, 2 * i, lambda st=st: v_chunk(st))
        for i, st in enumerate(range(8, NST)):
            F(2, 2 * i, lambda st=st: v_chunk(st))
        # remaining q chunks (c2 late in ci1, c3 early in ci2 so the proj
        # pool can close at ci2 kt6)
        F(1, 11, lambda: qk_chunk(qhT[0], wq_s, xq_t, 0, 2, bias=True))
        F(1, 13, lambda: qk_chunk(qhT[1], wq_s, xq_t, 1, 2, bias=True))
        F(2, 1, lambda: qk_chunk(qhT[0], wq_s, xq_t, 0, 3, bias=True))
        F(2, 3, lambda: qk_chunk(qhT[1], wq_s, xq_t, 1, 3, bias=True))
        # out-projection quanta: qb0 -> ci4, qb1 -> ci6 (qb2/qb3 in tail)
        for i in range(4):
            F(4, 2 * i + 5, lambda i=i: outproj_quantum(0, i))
            F(6, 2 * i + 5, lambda i=i: outproj_quantum(1, i))

        # ---------------- main pipeline ----------------------------------
        for ci, (qb, hp) in enumerate(combos):
            et = expT_pool.tile([128, NST, 2, 512], BF16, tag="expT", name=f"expT{ci}")
            expT[ci] = et
            for kt in range(NST):
                sc = sc_pool.tile([128, 2, 512], F32, tag="sc", name=f"sc{ci}_{kt}")
                for e in range(2):
                    po = e * 64
                    nc.tensor.matmul(
                        sc[:, e, :],
                        lhsT=khT[hp][po : po + 64, kt * 128 : (kt + 1) * 128],
                        rhs=qhT[hp][po : po + 64, qb * 512 : (qb + 1) * 512],
                        start=True,
                        stop=True,
                        tile_position=(po, 0),
                    )
                nc.scalar.activation(et[:, kt, :, :], sc, AF.Exp)
                if kt == 1 and ci >= LAG + 1:
                    bcast_mm(ci - LAG - 1)
                if kt == 2 and ci >= LAG + 1:
                    normalize(ci - LAG - 1)
                if kt >= PV_OFF and ci >= LAG:
                    pv_step(ci - LAG, kt - PV_OFF)
                for fn in fill.pop((ci, kt), ()):
                    fn()
                if ci == 2 and kt == 15:
                    # all projection work emitted; swap proj PSUM for
                    # out-proj + broadcast
                    proj_stack.close()
                    outp_pool[0] = ctx.enter_context(
                        tc.tile_pool(name="out_psum", bufs=2, space="PSUM")
                    )
            if ci >= LAG:
                for kt in range(NST - PV_OFF, NST):
                    pv_step(ci - LAG, kt)
                stats(ci - LAG)

        # ---------------- tail -------------------------------------------
        bcast_mm(NC - 3)
        normalize(NC - 3)
        for ci in (NC - 2, NC - 1):
            for kt in range(NST):
                pv_step(ci, kt)
                if ci == NC - 2 and kt in (5, 7, 9, 11):
                    outproj_quantum(2, (kt - 5) // 2)
            stats(ci)
            bcast_mm(ci)
            normalize(ci)
        for stl in range(4):
            outproj_quantum(3, stl)

    nc.compile()
    return nc


_CACHE: dict = {}


def _get_nc() -> bass.Bass:
    if "nc" not in _CACHE:
        _CACHE["nc"] = build_nc()
    return _CACHE["nc"]


def _bf16(x: np.ndarray) -> np.ndarray:
    return np.ascontiguousarray(x).astype(ml_dtypes.bfloat16)


def make_in_maps(q, k, v, wq, bq, wk, bk, wv, bv, wo, bo):
    scale = np.float32(1.0 / np.sqrt(DK))
    in_maps = []
    for c in range(NCORES):
        b, g = divmod(c, 4)
        hh = g * HD
        in_maps.append(
            {
                "xqT": _bf16(np.asarray(q[b], np.float32).T),
                "xkT": _bf16(np.asarray(k[b], np.float32).T),
                "xvT": _bf16(np.asarray(v[b], np.float32).T),
                "wqT": _bf16(np.asarray(wq[hh : hh + HD], np.float32).T * scale),
                "wkT": _bf16(np.asarray(wk[hh : hh + HD], np.float32).T),
                "wvT": _bf16(np.asarray(wv[hh : hh + HD], np.float32).T),
                "woT": _bf16(np.asarray(wo[:, hh : hh + HD], np.float32).T),
                "bq": _bf16(np.asarray(bq[hh : hh + HD], np.float32) * scale),
            }
        )
    return in_maps


def assemble(results, bv, bo, wo) -> np.ndarray:
    out = np.zeros((B, S, D), np.float32)
    for c in range(NCORES):
        out[c // 4] += np.asarray(results[c]["out_partial"], np.float32)
    corr = np.asarray(bv, np.float32) @ np.asarray(wo, np.float32).T + np.asarray(
        bo, np.float32
    )
    out += corr[None, None, :]
    return out


def kernel(q, k, v, wq, bq, wk, bk, wv, bv, wo, bo) -> np.ndarray:
    nc = _get_nc()
    in_maps = make_in_maps(q, k, v, wq, bq, wk, bk, wv, bv, wo, bo)
    res = run_bass_kernel_spmd(nc, in_maps, list(range(NCORES))).results
    return assemble(res, bv, bo, wo)


# revision 17
# speedup vs baseline: 1.4492x; 1.0035x over previous
"""Multi-head attention (B=2, S=2048, D=1024, H=16, dk=64) on 8 trn2 cores.

Sharding: core c handles batch b=c//4 and 4 heads g=c%4 (heads 4g..4g+3).
Each core computes its heads' Q/K/V projections, attention, and a partial
output projection; the host sums the 4 partials per batch.

v3 — transposed-scores layout (k on partitions), denominator fused into PV:
  - qhT/khT [128 = 2 heads x 64 dk, S] bf16; vh [128 k, 16 kt, 4 h, 65]
    bf16 with a ones column at index 64.
  - scoresT [128 k, 512 q] per (head, kt): lhsT = khT slice (K=64 rows at
    partition base po), rhs = qhT slice. The two heads of a pair are
    row-packed (tile_position (0,0)/(64,0)) and run concurrently on the PE.
  - exp on ScalarE from PSUM [128, 2, 2, 512] (one kt pair, both heads) to
    bf16 expT in SBUF. No max subtraction (scores ~ N(0,1)).
  - PV contracts k on partitions: lhsT = [vh | ones] [128, 65], rhs = expT
    slice; head e accumulates into its own ctx PSUM bank [0:65, e, :] over
    the 16 kt. Row 64 is the softmax denominator — computed by the PE for
    free, in fp32, consistent with the numerator.
  - per combo: reciprocal_approx_fast on the denominator row, cast to bf16,
    partition-broadcast via K=1 PE matmuls (ones column) into an out-proj
    PSUM tile, and the normalize-multiply is fused into the ctx PSUM->SBUF
    copy on DVE.
  - out projection reads ctxT (already [hd, q]) against resident wo.
  - bk/bv/bo never touch the device: bk shifts scores by a per-row constant
    (softmax-invariant), bv/bo fold into a host-side correction, bq is a
    rank-1 matmul on-device (x 1/sqrt(dk), folded into wq/bq on the host).

Emission is software-pipelined so ScalarE (~126us of exp) streams: combo ci
emits scores(ci), PV(ci-2) offset by 3 kt steps (so the normalize chain of
the previous ctx tile never stalls the PE), and filler quanta (projection
chunks / out-proj quanta). PSUM: sc(4 banks) + ctx(2) + proj(2, bufs=2) = 8;
the out-proj/broadcast pool (2) replaces the proj pool at combo 2.
"""

import sys

for _p in ("/opt/trn_rl_repo",):
    if _p not in sys.path:
        sys.path.insert(0, _p)

from contextlib import ExitStack

import ml_dtypes
import numpy as np

import concourse.bass as bass
import concourse.bacc as bacc_mod
import concourse.mybir as mybir
import concourse.tile as tile
from concourse.bass_utils import run_bass_kernel_spmd

BF16 = mybir.dt.bfloat16
F32 = mybir.dt.float32
AF = mybir.ActivationFunctionType

B, S, D = 2, 2048, 1024
NCORES = 8
HLOC = 4          # heads per core
DK = 64
HD = HLOC * DK    # local head dims = 256
KT_D = D // 128   # 8 k-tiles over the model dim
NST = S // 128    # 16 tiles over sequence (kv)
NQB = 4           # q blocks of 512
NCH = 4           # proj output chunks of 512 over S
LAG = 1           # PV trails scores by LAG combos
PV_OFF = 4        # PV step kt runs at parent step kt+PV_OFF


def build_nc() -> bass.Bass:
    nc = bacc_mod.Bacc()

    xqT = nc.dram_tensor("xqT", [D, S], BF16, kind="ExternalInput")
    xkT = nc.dram_tensor("xkT", [D, S], BF16, kind="ExternalInput")
    xvT = nc.dram_tensor("xvT", [D, S], BF16, kind="ExternalInput")
    wqT = nc.dram_tensor("wqT", [D, HD], BF16, kind="ExternalInput")
    wkT = nc.dram_tensor("wkT", [D, HD], BF16, kind="ExternalInput")
    wvT = nc.dram_tensor("wvT", [D, HD], BF16, kind="ExternalInput")
    woT = nc.dram_tensor("woT", [HD, D], BF16, kind="ExternalInput")
    bqd = nc.dram_tensor("bq", [HD], BF16, kind="ExternalInput")
    out = nc.dram_tensor("out_partial", [S, D], F32, kind="ExternalOutput")

    with tile.TileContext(nc) as tc, ExitStack() as ctx:
        const = ctx.enter_context(tc.tile_pool(name="const", bufs=1))
        persist = ctx.enter_context(tc.tile_pool(name="persist", bufs=1))

        wq_s = const.tile([128, KT_D, HD], BF16, tag="wq")
        wk_s = const.tile([128, KT_D, HD], BF16, tag="wk")
        wv_s = const.tile([128, KT_D, HD], BF16, tag="wv")
        wo_s = const.tile([128, 2, D], BF16, tag="wo")
        bq_s = const.tile([1, HD], BF16, tag="bq")
        ones_row = const.tile([1, 512], BF16, tag="ones")
        ones_col = const.tile([1, DK], BF16, tag="onesc")
        nc.vector.memset(ones_row, 1.0)
        nc.vector.memset(ones_col, 1.0)

        # sync queue: wk, wq, bq, xq chunks..., wv, wo. gpsimd queue: xk
        # chunks, xv chunks, output stores.
        nc.sync.dma_start(wk_s, wkT.rearrange("(t p) n -> p t n", p=128))
        nc.sync.dma_start(wq_s, wqT.rearrange("(t p) n -> p t n", p=128))
        nc.sync.dma_start(bq_s, bqd.rearrange("(o n) -> o n", o=1))

        qhT = [persist.tile([128, S], BF16, tag=f"qhT{m}", name=f"qhT{m}") for m in range(2)]
        khT = [persist.tile([128, S], BF16, tag=f"khT{m}", name=f"khT{m}") for m in range(2)]
        # vh with a ones column per (kt, head): PV computes the softmax
        # denominator into ctx row 64 for free.
        vh_s = persist.tile([128, NST, HLOC, DK + 1], BF16, tag="vh")
        nc.vector.memset(vh_s[:, :, :, DK], 1.0)
        ctxT = [persist.tile([128, S], BF16, tag=f"ctxT{m}", name=f"ctxT{m}") for m in range(2)]

        xkp = ctx.enter_context(tc.tile_pool(name="xkload", bufs=2))
        xqp = ctx.enter_context(tc.tile_pool(name="xqload", bufs=2))
        xvp = ctx.enter_context(tc.tile_pool(name="xvload", bufs=2))
        xk_t, xq_t, xv_t = {}, {}, {}

        def load_xk(c, split=False):
            t = xkp.tile([128, KT_D, 512], BF16, tag="xk", name=f"xk{c}")
            if split:
                for hf in range(2):
                    nc.gpsimd.dma_start(
                        t[:, :, hf * 256 : (hf + 1) * 256],
                        xkT[:, c * 512 + hf * 256 : c * 512 + (hf + 1) * 256].rearrange(
                            "(t p) n -> p t n", p=128
                        ),
                    )
            else:
                nc.gpsimd.dma_start(
                    t, xkT[:, c * 512 : (c + 1) * 512].rearrange("(t p) n -> p t n", p=128)
                )
            xk_t[c] = t

        def load_xq(c, split=False):
            t = xqp.tile([128, KT_D, 512], BF16, tag="xq", name=f"xq{c}")
            if split:
                for hf in range(2):
                    nc.sync.dma_start(
                        t[:, :, hf * 256 : (hf + 1) * 256],
                        xqT[:, c * 512 + hf * 256 : c * 512 + (hf + 1) * 256].rearrange(
                            "(t p) n -> p t n", p=128
                        ),
                    )
            else:
                nc.sync.dma_start(
                    t, xqT[:, c * 512 : (c + 1) * 512].rearrange("(t p) n -> p t n", p=128)
                )
            xq_t[c] = t

        def load_xv(pr):
            t = xvp.tile([128, KT_D, 256], BF16, tag="xv", name=f"xv{pr}")
            nc.gpsimd.dma_start(
                t, xvT[:, pr * 256 : (pr + 1) * 256].rearrange("(t p) n -> p t n", p=128)
            )
            xv_t[pr] = t

        load_xk(0, split=True)
        load_xk(1)
        load_xq(0, split=True)
        load_xq(1)
        load_xv(0)
        load_xv(1)
        nc.sync.dma_start(wv_s, wvT.rearrange("(t p) n -> p t n", p=128))
        nc.sync.dma_start(wo_s, woT.rearrange("(t p) n -> p t n", p=128))

        # softmax stat tiles
        denf = persist.tile([1, 2, 512], F32, tag="denf")
        recipf = persist.tile([1, 512], F32, tag="recipf")
        recip = [persist.tile([1, 512], BF16, tag=f"rec{e}", name=f"rec{e}") for e in range(2)]
        bcs = persist.tile([128, 512], BF16, tag="bcs")

        expT_pool = ctx.enter_context(tc.tile_pool(name="expT", bufs=LAG + 1))
        ostage_pool = ctx.enter_context(tc.tile_pool(name="ostage", bufs=2))

        sc_pool = ctx.enter_context(tc.tile_pool(name="sc_psum", bufs=2, space="PSUM"))
        cb_pool = ctx.enter_context(tc.tile_pool(name="cb_psum", bufs=1, space="PSUM"))

        proj_stack = ExitStack()
        qkp = proj_stack.enter_context(tc.tile_pool(name="qk_psum", bufs=1, space="PSUM"))

        # ---------------- emission helpers -------------------------------
        def qk_chunk(dst, w_s, xt, m, c, bias, after=None):
            ps = qkp.tile([128, 512], F32, tag="proj")
            for kt in range(KT_D):
                nc.tensor.matmul(
                    ps,
                    lhsT=w_s[:, kt, m * 128 : (m + 1) * 128],
                    rhs=xt[c][:, kt, :],
                    start=(kt == 0),
                    stop=(kt == KT_D - 1) and not bias,
                )
            if bias:
                nc.tensor.matmul(
                    ps,
                    lhsT=bq_s[0:1, m * 128 : (m + 1) * 128],
                    rhs=ones_row[0:1, :],
                    start=False,
                    stop=True,
                )
            nc.vector.tensor_copy(dst[:, c * 512 : (c + 1) * 512], ps)
            if after is not None:
                after()

        def qk_half(dst, w_s, xt, m, c, hf, bias):
            ps = qkp.tile([128, 512], F32, tag="proj", name=f"ph{m}{c}{hf}")
            for kt in range(KT_D):
                nc.tensor.matmul(
                    ps[:, 0:256],
                    lhsT=w_s[:, kt, m * 128 : (m + 1) * 128],
                    rhs=xt[c][:, kt, hf * 256 : (hf + 1) * 256],
                    start=(kt == 0),
                    stop=(kt == KT_D - 1) and not bias,
                )
            if bias:
                nc.tensor.matmul(
                    ps[:, 0:256],
                    lhsT=bq_s[0:1, m * 128 : (m + 1) * 128],
                    rhs=ones_row[0:1, 0:256],
                    start=False,
                    stop=True,
                )
            nc.vector.tensor_copy(
                dst[:, c * 512 + hf * 256 : c * 512 + (hf + 1) * 256], ps[:, 0:256]
            )

        def v_chunk(st):
            ps = qkp.tile([128, 512], F32, tag="proj", name=f"vproj{st}")
            pr, half = st // 2, st % 2
            for kt in range(KT_D):
                nc.tensor.matmul(
                    ps[:, 0:HD],
                    lhsT=xv_t[pr][:, kt, half * 128 : (half + 1) * 128],
                    rhs=wv_s[:, kt, :],
                    start=(kt == 0),
                    stop=(kt == KT_D - 1),
                )
            nc.vector.tensor_copy(
                vh_s[:, st, :, 0:DK],
                ps[:, 0:HD].rearrange("p (h d) -> p h d", h=HLOC),
            )
            if half == 1 and pr + 2 < NST // 2:
                load_xv(pr + 2)

        combos = [(qb, hp) for qb in range(NQB) for hp in range(2)]
        NC = len(combos)
        expT, ctx_ps = {}, {}

        def stats(ci):
            """1/denominator from ctx row 64, per head (after PV(ci))."""
            for e in range(2):
                # stage the PSUM denominator row through SBUF: the custom-DVE
                # reciprocal misreads PSUM for some values on HW
                nc.vector.tensor_copy(denf[:, e, :], ctx_ps[ci][64:65, e, :])
            for e in range(2):
                nc.vector.reciprocal_approx_fast(recipf, denf[:, e, :])
                nc.vector.tensor_copy(recip[e], recipf)

        def bcast_mm(ci):
            bc = cb_pool.tile([128, 512], F32, tag="bcast", name=f"bc{ci}")
            for e in range(2):
                nc.tensor.matmul(
                    bc[e * 64 : (e + 1) * 64, :],
                    lhsT=ones_col[0:1, :],
                    rhs=recip[e][0:1, :],
                    start=True,
                    stop=True,
                    tile_position=(0, e * 64),
                )
            nc.vector.tensor_copy(bcs, bc)

        def normalize(ci):
            qb, hp = combos[ci]
            cp = ctx_ps[ci]
            for e in range(2):
                sl = slice(e * 64, (e + 1) * 64)
                nc.vector.tensor_mul(
                    ctxT[hp][sl, qb * 512 : (qb + 1) * 512], cp[0:64, e, :], bcs[sl, :]
                )
            del expT[ci], ctx_ps[ci]

        def pv_step(ci, kt):
            qb, hp = combos[ci]
            if kt == 0:
                ctx_ps[ci] = cb_pool.tile([128, 2, 512], F32, tag="ctx", name=f"ctx{ci}")
            for e in range(2):
                nc.tensor.matmul(
                    ctx_ps[ci][0 : DK + 1, e, :],
                    lhsT=vh_s[:, kt, hp * 2 + e, :],
                    rhs=expT[ci][:, kt, e, :],
                    start=(kt == 0),
                    stop=(kt == NST - 1),
                )

        outp_pool = [None]

        def outproj_quantum(qb, stl):
            st = qb * 4 + stl
            outp = outp_pool[0]
            for nb in range(2):
                ps = outp.tile([128, 512], F32, tag="op", name=f"op{st}_{nb}")
                for kt2 in range(2):
                    nc.tensor.matmul(
                        ps,
                        lhsT=ctxT[kt2][:, st * 128 : (st + 1) * 128],
                        rhs=wo_s[:, kt2, nb * 512 : (nb + 1) * 512],
                        start=(kt2 == 0),
                        stop=(kt2 == 1),
                    )
                ob = ostage_pool.tile([128, 512], F32, tag="ob")
                nc.vector.tensor_copy(ob, ps)
                nc.gpsimd.dma_start(
                    out[st * 128 : (st + 1) * 128, nb * 512 : (nb + 1) * 512], ob
                )

        # ---------------- prologue ---------------------------------------
        # khT[0]/qhT[0] chunk-0 halves: combo 0's first score kt-tiles only
        # need 256 columns of each.
        qk_half(khT[0], wk_s, xk_t, 0, 0, 0, bias=False)
        qk_half(khT[0], wk_s, xk_t, 0, 0, 1, bias=False)
        qk_half(qhT[0], wq_s, xq_t, 0, 0, 0, bias=True)
        qk_half(qhT[0], wq_s, xq_t, 0, 0, 1, bias=True)

        # Filler schedule: {(ci, kt): [thunk, ...]}.  K chunk c feeds score
        # kt-tiles 4c..4c+3.
        fill = {}

        def F(ci, kt, fn):
            fill.setdefault((ci, kt), []).append(fn)

        # ci0 even steps: remaining khT chunks + qhT[1] c0 (deadlines:
        # Km0c_n before step 4n; all m1 chunks + Qm1c0 before combo 1)
        F(0, 0, lambda: qk_chunk(khT[0], wk_s, xk_t, 0, 1, bias=False))
        F(0, 2, lambda: qk_chunk(khT[1], wk_s, xk_t, 1, 0, bias=False,
                                 after=lambda: load_xk(2)))
        F(0, 4, lambda: qk_chunk(khT[0], wk_s, xk_t, 0, 2, bias=False))
        F(0, 6, lambda: qk_chunk(khT[1], wk_s, xk_t, 1, 1, bias=False,
                                 after=lambda: load_xk(3)))
        F(0, 8, lambda: qk_chunk(khT[0], wk_s, xk_t, 0, 3, bias=False))
        F(0, 10, lambda: qk_chunk(khT[1], wk_s, xk_t, 1, 2, bias=False))
        F(0, 12, lambda: qk_chunk(qhT[1], wq_s, xq_t, 1, 0, bias=True,
                                  after=lambda: load_xq(2)))
        F(0, 14, lambda: qk_chunk(khT[1], wk_s, xk_t, 1, 3, bias=False))
        # V projection: st 0..7 on ci0 odd steps, st 8..15 on ci1 even
        # steps (PV(0) at combo 1 consumes st_kt at step kt+PV_OFF).
        for i, st in enumerate(range(8)):
            F(0, 2 * i + 1, lambda st=st: v_chunk(st))
        for i, st in enumerate(range(8, NST)):
            F(1, 2 * i, lambda st=st: v_chunk(st))
        # remaining q chunks: c1 in ci1, c2 in ci2, c3 in ci3
        F(1, 11, lambda: qk_chunk(qhT[0], wq_s, xq_t, 0, 1, bias=True))
        F(1, 13, lambda: qk_chunk(qhT[1], wq_s, xq_t, 1, 1, bias=True))
        F(2, 1, lambda: qk_chunk(qhT[0], wq_s, xq_t, 0, 2, bias=True,
                                 after=lambda: load_xq(3)))
        F(2, 3, lambda: qk_chunk(qhT[1], wq_s, xq_t, 1, 2, bias=True))
        F(3, 1, lambda: qk_chunk(qhT[0], wq_s, xq_t, 0, 3, bias=True))
        F(3, 3, lambda: qk_chunk(qhT[1], wq_s, xq_t, 1, 3, bias=True))
        # out-projection quanta: qb0 -> ci4, qb1 -> ci6, qb2 -> ci7
        for i in range(4):
            F(4, 2 * i + 5, lambda i=i: outproj_quantum(0, i))
            F(6, 2 * i + 5, lambda i=i: outproj_quantum(1, i))
            F(7, 2 * i + 5, lambda i=i: outproj_quantum(2, i))

        # ---------------- main pipeline ----------------------------------
        for ci, (qb, hp) in enumerate(combos):
            et = expT_pool.tile([128, NST, 2, 512], BF16, tag="expT", name=f"expT{ci}")
            expT[ci] = et
            for kt in range(NST):
                sc = sc_pool.tile([128, 2, 512], F32, tag="sc", name=f"sc{ci}_{kt}")
                for e in range(2):
                    po = e * 64
                    nc.tensor.matmul(
                        sc[:, e, :],
                        lhsT=khT[hp][po : po + 64, kt * 128 : (kt + 1) * 128],
                        rhs=qhT[hp][po : po + 64, qb * 512 : (qb + 1) * 512],
                        start=True,
                        stop=True,
                        tile_position=(po, 0),
                    )
                nc.scalar.activation(et[:, kt, :, :], sc, AF.Exp)
                if kt == 2 and ci >= LAG + 1:
                    bcast_mm(ci - LAG - 1)
                if kt == 3 and ci >= LAG + 1:
                    normalize(ci - LAG - 1)
                if kt >= PV_OFF and ci >= LAG:
                    pv_step(ci - LAG, kt - PV_OFF)
                for fn in fill.pop((ci, kt), ()):
                    fn()
                if ci == 3 and kt == 4:
                    # all projection work emitted; swap proj PSUM for out-proj
                    proj_stack.close()
                    outp_pool[0] = ctx.enter_context(
                        tc.tile_pool(name="out_psum", bufs=1, space="PSUM")
                    )
            if ci >= LAG:
                for kt in range(NST - PV_OFF, NST):
                    pv_step(ci - LAG, kt)
                stats(ci - LAG)

        # ---------------- tail -------------------------------------------
        bcast_mm(NC - 2)
        normalize(NC - 2)
        ci = NC - 1
        for kt in range(NST):
            pv_step(ci, kt)
        stats(ci)
        bcast_mm(ci)
        normalize(ci)
        for stl in range(4):
            outproj_quantum(3, stl)

    nc.compile()
    return nc


_CACHE: dict = {}


def _get_nc() -> bass.Bass:
    if "nc" not in _CACHE:
        _CACHE["nc"] = build_nc()
    return _CACHE["nc"]


def _bf16(x: np.ndarray) -> np.ndarray:
    return np.ascontiguousarray(x).astype(ml_dtypes.bfloat16)


def make_in_maps(q, k, v, wq, bq, wk, bk, wv, bv, wo, bo):
    scale = np.float32(1.0 / np.sqrt(DK))
    in_maps = []
    for c in range(NCORES):
        b, g = divmod(c, 4)
        hh = g * HD
        in_maps.append(
            {
                "xqT": _bf16(np.asarray(q[b], np.float32).T),
                "xkT": _bf16(np.asarray(k[b], np.float32).T),
                "xvT": _bf16(np.asarray(v[b], np.float32).T),
                "wqT": _bf16(np.asarray(wq[hh : hh + HD], np.float32).T * scale),
                "wkT": _bf16(np.asarray(wk[hh : hh + HD], np.float32).T),
                "wvT": _bf16(np.asarray(wv[hh : hh + HD], np.float32).T),
                "woT": _bf16(np.asarray(wo[:, hh : hh + HD], np.float32).T),
                "bq": _bf16(np.asarray(bq[hh : hh + HD], np.float32) * scale),
            }
        )
    return in_maps


def assemble(results, bv, bo, wo) -> np.ndarray:
    out = np.zeros((B, S, D), np.float32)
    for c in range(NCORES):
        out[c // 4] += np.asarray(results[c]["out_partial"], np.float32)
    corr = np.asarray(bv, np.float32) @ np.asarray(wo, np.float32).T + np.asarray(
        bo, np.float32
    )
    out += corr[None, None, :]
    return out


def kernel(q, k, v, wq, bq, wk, bk, wv, bv, wo, bo) -> np.ndarray:
    nc = _get_nc()
    in_maps = make_in_maps(q, k, v, wq, bq, wk, bk, wv, bv, wo, bo)
    res = run_bass_kernel_spmd(nc, in_maps, list(range(NCORES))).results
    return assemble(res, bv, bo, wo)
